# revision 1
# baseline (speedup 1.0000x reference)
"""Trainium2 Bass kernel for nn_DecoderBlock (B=2, T=2048, D=1024, H=16, MLP=4096).

Sharding: sequence/row parallel over 8 cores (4 cores per batch, 512 rows each).
K/V for both attentions are computed on local rows and AllGathered (2 groups of
4 cores, one group per batch element). Everything else is row-local.

On-device layout: activations transposed [feature, row] so matmuls need no
transposes; LayerNorm affine params are folded into the following weight
matrix on the host (exact algebra). Matmul operands bf16, accumulation fp32,
residual stream fp32. Softmax without max-subtraction (logits bounded ~|2.5|),
mask applied as 0/1 multiply on exp(scores); denominator comes from a ones
column appended to V in the PV matmul.
"""

import sys

if "/opt/trn_rl_repo" not in sys.path:
    sys.path.insert(0, "/opt/trn_rl_repo")

import numpy as np
import ml_dtypes

import concourse.bass as bass
import concourse.mybir as mybir
import concourse.tile as tile
from concourse import bacc
from concourse.bass_utils import run_bass_kernel_spmd

F32 = mybir.dt.float32
F32R = mybir.dt.float32r
BF16 = mybir.dt.bfloat16

B, T, D, H, HD = 2, 2048, 1024, 16, 64
MLP = 4 * D
EPS = 1e-5
N_CORES = 8
GROUP = 4            # cores per batch element
R = T // GROUP       # rows per core = 512
DC = D // 128        # feature chunks = 8
MC = MLP // 128      # mlp chunks = 32
KC = T // 128        # key chunks = 16
NPAIR = H // 2       # head pairs = 8
SCALE = HD ** -0.5


def _r32(ap):
    return ap.bitcast(F32R)


def build_program(trace_scopes=False):
    nc = bacc.Bacc("TRN2", target_bir_lowering=False, debug=False,
                   num_devices=N_CORES)

    # ---- DRAM I/O ----
    x_t = nc.dram_tensor("x_t", [D, R], F32, kind="ExternalInput")
    enc_tb = nc.dram_tensor("enc_tb", [D, R], BF16, kind="ExternalInput")
    mask_t = nc.dram_tensor("mask_t", [T, R], BF16, kind="ExternalInput")
    wq = nc.dram_tensor("wq", [DC, DC, 128, 128], BF16, kind="ExternalInput")
    wk = nc.dram_tensor("wk", [DC, DC, 128, 128], BF16, kind="ExternalInput")
    wv = nc.dram_tensor("wv", [DC, 128, D], BF16, kind="ExternalInput")
    wproj = nc.dram_tensor("wproj", [DC, DC, 128, 128], BF16, kind="ExternalInput")
    wq2 = nc.dram_tensor("wq2", [DC, DC, 128, 128], BF16, kind="ExternalInput")
    wkvk = nc.dram_tensor("wkvk", [DC, DC, 128, 128], BF16, kind="ExternalInput")
    wkvv = nc.dram_tensor("wkvv", [DC, 128, D], BF16, kind="ExternalInput")
    wco = nc.dram_tensor("wco", [DC, DC, 128, 128], BF16, kind="ExternalInput")
    wm1 = nc.dram_tensor("wm1", [DC, MC, 128, 128], BF16, kind="ExternalInput")
    wm2 = nc.dram_tensor("wm2", [MC, DC, 128, 128], BF16, kind="ExternalInput")
    bq = nc.dram_tensor("bq", [DC, 128], F32, kind="ExternalInput")
    bk = nc.dram_tensor("bk", [DC, 128], F32, kind="ExternalInput")
    bv = nc.dram_tensor("bv", [1, D], BF16, kind="ExternalInput")
    bproj = nc.dram_tensor("bproj", [DC, 128], F32, kind="ExternalInput")
    bq2 = nc.dram_tensor("bq2", [DC, 128], F32, kind="ExternalInput")
    bkvk = nc.dram_tensor("bkvk", [DC, 128], F32, kind="ExternalInput")
    bkvv = nc.dram_tensor("bkvv", [1, D], BF16, kind="ExternalInput")
    bco = nc.dram_tensor("bco", [DC, 128], F32, kind="ExternalInput")
    bm1 = nc.dram_tensor("bm1", [MC, 128], F32, kind="ExternalInput")
    bm2 = nc.dram_tensor("bm2", [DC, 128], F32, kind="ExternalInput")
    out_t = nc.dram_tensor("out_t", [D, R], F32, kind="ExternalOutput")

    rg = [[0, 1, 2, 3], [4, 5, 6, 7]]

    with tile.TileContext(nc) as tc:
        with (
            tc.tile_pool(name="persist", bufs=1) as pp,
            tc.tile_pool(name="dram", bufs=1, space="DRAM") as dram,
        ):
            # collective bounce buffers, split in head-halves so attention
            # can start after the first half's AllGather lands
            HH, DH = H // 2, D // 2
            ktS_in = [dram.tile([DH, R], BF16, name=f"ktSi{i}") for i in range(2)]
            vS_in = [dram.tile([HH, R, HD], BF16, name=f"vSi{i}") for i in range(2)]
            ktC_in = [dram.tile([DH, R], BF16, name=f"ktCi{i}") for i in range(2)]
            vC_in = [dram.tile([HH, R, HD], BF16, name=f"vCi{i}") for i in range(2)]
            ktS_out = [dram.tile([GROUP * DH, R], BF16, name=f"ktSo{i}") for i in range(2)]
            vS_out = [dram.tile([GROUP * HH, R, HD], BF16, name=f"vSo{i}") for i in range(2)]
            ktC_out = [dram.tile([GROUP * DH, R], BF16, name=f"ktCo{i}") for i in range(2)]
            vC_out = [dram.tile([GROUP * HH, R, HD], BF16, name=f"vCo{i}") for i in range(2)]

            def allgather(src_d, dst_d):
                nc.gpsimd.collective_compute(
                    "AllGather", mybir.AluOpType.bypass,
                    ins=[src_d.opt()], outs=[dst_d.opt()], replica_groups=rg)

            # persistent SBUF
            x_sb = pp.tile([128, DC, R], F32)        # residual stream x^T
            x2_sb = pp.tile([128, DC, R], F32)
            x3_sb = pp.tile([128, DC, R], F32)
            enc_sb = pp.tile([128, DC, R], BF16)
            mask_sb = pp.tile([128, KC, R], BF16)
            qt_sb = pp.tile([128, DC, R], BF16)      # Q^T (self)
            qt2_sb = pp.tile([128, DC, R], BF16)     # Q^T (cross)
            at_self = pp.tile([128, DC, R], BF16)    # attn out^T (self)
            at_cross = pp.tile([128, DC, R], BF16)   # attn out^T (cross)
            ones_full = pp.tile([128, 128], F32)     # lhsT for sums/bcasts
            ones_rbf = pp.tile([1, 128], BF16)       # lhsT for V bias rank-1
            ones_bf_col = pp.tile([128, 1], BF16)    # lhsT for bf16 partition sums
            bias_sb = pp.tile([128, 8 * DC + MC], F32)
            bv_sb = pp.tile([1, D], BF16)
            bkvv_sb = pp.tile([1, D], BF16)

            eps_sb = pp.tile([1, 1], F32)
            nc.vector.memset(ones_full[:], 1.0)
            nc.vector.memset(ones_rbf[:], 1.0)
            nc.vector.memset(ones_bf_col[:], 1.0)
            nc.vector.memset(eps_sb[:], EPS)

            nc.sync.dma_start(x_sb[:], x_t.ap().rearrange("(c p) f -> p c f", p=128))
            nc.sync.dma_start(enc_sb[:], enc_tb.ap().rearrange("(c p) f -> p c f", p=128))
            nc.sync.dma_start(mask_sb[:], mask_t.ap().rearrange("(c p) f -> p c f", p=128))
            nc.sync.dma_start(bv_sb[:], bv.ap())
            nc.sync.dma_start(bkvv_sb[:], bkvv.ap())
            # biases: 8 arrays of [DC,128] then bm1 [MC,128]
            bias_list = [bq, bk, bproj, bq2, bkvk, bco, bm2]
            for i, b in enumerate(bias_list):
                nc.sync.dma_start(
                    bias_sb[:, i * DC:(i + 1) * DC],
                    b.ap().rearrange("c p -> p c"),
                )
            nc.sync.dma_start(
                bias_sb[:, 7 * DC:7 * DC + MC], bm1.ap().rearrange("c p -> p c")
            )
            B_Q, B_K, B_PROJ, B_Q2, B_KVK, B_CO, B_M2 = (
                0, DC, 2 * DC, 3 * DC, 4 * DC, 5 * DC, 6 * DC)
            B_M1 = 7 * DC

            def bias_ap(base, oc):
                return bias_sb[:, base + oc:base + oc + 1]

            # ---------- helpers ----------
            def layernorm(src_sb, pool, psum_pool, name):
                """src_sb [128, DC, R] f32 -> ln^T bf16 [128, DC, R] (no affine)."""
                ps1 = psum_pool.tile([1, R], F32, tag="stats", bufs=2,
                                     name=f"p1_{name}")
                ps2 = psum_pool.tile([1, R], F32, tag="stats", bufs=2,
                                     name=f"p2_{name}")
                for c in range(DC):
                    sq = pool.tile([128, R], BF16, tag="lnsq", name=f"sq_{name}{c}")
                    xb = pool.tile([128, R], BF16, tag="lnxb", name=f"xb_{name}{c}")
                    nc.scalar.square(sq[:], src_sb[:, c, :])
                    nc.vector.tensor_copy(xb[:], src_sb[:, c, :])
                    nc.tensor.matmul(ps1[:], lhsT=ones_bf_col[:],
                                     rhs=xb[:],
                                     start=(c == 0), stop=(c == DC - 1))
                    nc.tensor.matmul(ps2[:], lhsT=ones_bf_col[:],
                                     rhs=sq[:],
                                     start=(c == 0), stop=(c == DC - 1))
                nmean = pool.tile([1, R], F32, tag="lnrow", bufs=8, name=f"nm_{name}")
                ex2 = pool.tile([1, R], F32, tag="lnrow", bufs=8, name=f"e2_{name}")
                m2 = pool.tile([1, R], F32, tag="lnrow", bufs=8, name=f"m2_{name}")
                var = pool.tile([1, R], F32, tag="lnrow", bufs=8, name=f"va_{name}")
                std = pool.tile([1, R], F32, tag="lnrow", bufs=8, name=f"sd_{name}")
                rstd = pool.tile([1, R], F32, tag="lnrow", bufs=8, name=f"rs_{name}")
                nmrs = pool.tile([1, R], F32, tag="lnrow", bufs=8, name=f"nr_{name}")
                nc.scalar.activation(nmean[:], ps1[:],
                                     mybir.ActivationFunctionType.Identity,
                                     scale=-1.0 / D)
                nc.scalar.activation(ex2[:], ps2[:],
                                     mybir.ActivationFunctionType.Identity,
                                     scale=1.0 / D)
                nc.vector.tensor_tensor(m2[:], nmean[:], nmean[:],
                                        mybir.AluOpType.mult)
                nc.vector.tensor_tensor(var[:], ex2[:], m2[:],
                                        mybir.AluOpType.subtract)
                nc.scalar.activation(std[:], var[:],
                                     mybir.ActivationFunctionType.Sqrt,
                                     bias=eps_sb[:])
                nc.vector.reciprocal(rstd[:], std[:])
                nc.vector.tensor_tensor(nmrs[:], nmean[:], rstd[:],
                                        mybir.AluOpType.mult)
                # broadcast rstd / nmean*rstd across partitions via PE
                psb = psum_pool.tile([128, 2 * R], F32, tag="lnb", bufs=1,
                                     name=f"pb_{name}")
                nc.tensor.matmul(psb[:, 0:R], lhsT=ones_full[0:1, :],
                                 rhs=rstd[:], start=True, stop=True)
                nc.tensor.matmul(psb[:, R:2 * R], lhsT=ones_full[0:1, :],
                                 rhs=nmrs[:], start=True, stop=True)
                out = pool.tile([128, DC, R], BF16, tag="lnT", name=f"ln_{name}")
                tmp = pool.tile([128, R], F32, tag="lntmp", name=f"tp_{name}")
                for c in range(DC):
                    nc.vector.tensor_tensor(tmp[:], src_sb[:, c, :], psb[:, 0:R],
                                            mybir.AluOpType.mult)
                    nc.vector.tensor_tensor(out[:, c, :], tmp[:], psb[:, R:2 * R],
                                            mybir.AluOpType.add)
                return out

            def matmul_t(rhs_sb, w_dram, n_k, n_o, pool, psum_pool, name,
                         consume, w_tag="wtile", ocs=None):
                """Transposed-layout dense: out^T[oc] = sum_kc W[kc,oc].T @ rhs[kc].

                rhs_sb: [128, n_k, R] bf16.  w_dram: [n_k, n_o, 128, 128] bf16.
                consume(oc, psum_ap) handles each [128, R] output chunk.
                """
                for oc in (range(n_o) if ocs is None else ocs):
                    wt = pool.tile([128, n_k, 128], BF16, tag=w_tag,
                                   name=f"w_{name}_{oc}")
                    nc.gpsimd.dma_start(
                        wt[:], w_dram.ap()[:, oc].rearrange("k p m -> p k m"))
                    ps = psum_pool.tile([128, R], F32, tag="mm", name=f"ps_{name}_{oc}")
                    for kc in range(n_k):
                        nc.tensor.matmul(ps[:], lhsT=wt[:, kc, :],
                                         rhs=rhs_sb[:, kc, :],
                                         start=(kc == 0), stop=(kc == n_k - 1))
                    consume(oc, ps)

            def matmul_normal_v(lhs_sb, w_dram, bias_row, pool, psum_pool, name,
                                consume):
                """V = X @ Wv + b in normal layout, one 128-row chunk at a time.

                lhs_sb: [128, DC, R] bf16 (X^T chunks).  w_dram: [DC, 128, D].
                consume(rc, psum [128, D]).
                """
                for rc in range(R // 128):
                    ps = psum_pool.tile([128, D], F32, tag="pswide", bufs=1,
                                        name=f"ps_{name}_{rc}")
                    for kc in range(DC):
                        wt = pool.tile([128, D], BF16, tag="wv",
                                       name=f"wv_{name}_{rc}_{kc}")
                        nc.gpsimd.dma_start(wt[:], w_dram.ap()[kc])
                        for half in range(2):
                            sl = slice(half * 512, half * 512 + 512)
                            nc.tensor.matmul(
                                ps[:, sl],
                                lhsT=lhs_sb[:, kc, rc * 128:(rc + 1) * 128],
                                rhs=wt[:, sl],
                                start=(kc == 0), stop=False)
                    for half in range(2):
                        sl = slice(half * 512, half * 512 + 512)
                        nc.tensor.matmul(ps[:, sl], lhsT=ones_rbf[:],
                                         rhs=bias_row[:, sl],
                                         start=False, stop=True)
                    consume(rc, ps)

            def attention(qt, kt_halves, v_halves, dst_sb, masked, pool,
                          psum_pool, name):
                """dst_sb [128, DC, R] bf16 = attn(Q^T, gathered K^T/V)^T."""
                for hp in range(NPAIR):
                    half, hpl = hp // (NPAIR // 2), hp % (NPAIR // 2)
                    kt_out_d = kt_halves[half]
                    v_out_d = v_halves[half]
                    ktp = pool.tile([128, KC, 128], BF16, tag="ktp",
                                    name=f"kt_{name}_{hp}")
                    bpc = R // 128  # key chunks per gathered block = 4
                    for r in range(GROUP):
                        nc.sync.dma_start(
                            ktp[:, r * bpc:(r + 1) * bpc, :],
                            kt_out_d[r * DH + hpl * 128:(r * DH + (hpl + 1) * 128), :]
                            .rearrange("p (c m) -> p c m", m=128))
                    vts = []
                    for hh in range(2):
                        hl = 2 * hpl + hh
                        vt = pool.tile([128, KC, HD + 1], BF16, tag="vt", bufs=3,
                                       name=f"v_{name}_{2 * hp + hh}")
                        for r in range(GROUP):
                            nc.sync.dma_start(
                                vt[:, r * bpc:(r + 1) * bpc, 0:HD],
                                v_out_d[r * HH + hl]
                                .rearrange("(c p) d -> p c d", p=128))
                        nc.vector.memset(vt[:, :, HD:HD + 1], 1.0)
                        vts.append(vt)
                    psA = psum_pool.tile([HD + 1, R], F32, tag="psO", bufs=2,
                                         name=f"oA_{name}_{hp}")
                    psB = psum_pool.tile([HD + 1, R], F32, tag="psO", bufs=2,
                                         name=f"oB_{name}_{hp}")
                    for kc in range(KC):
                        pss = psum_pool.tile([128, 2 * R], F32, tag="psS", bufs=3,
                                             name=f"s_{name}_{hp}_{kc}")
                        nc.tensor.matmul(pss[:, 0:R],
                                         lhsT=ktp[0:64, kc, :],
                                         rhs=qt[0:64, hp, :],
                                         start=True, stop=True)
                        nc.tensor.matmul(pss[:, R:2 * R],
                                         lhsT=ktp[64:128, kc, :],
                                         rhs=qt[64:128, hp, :],
                                         start=True, stop=True)
                        es = pool.tile([128, 2, R], BF16, tag="expS",
                                       name=f"e_{name}_{hp}_{kc}")
                        nc.scalar.activation(
                            es[:].rearrange("p a f -> p (a f)"), pss[:],
                            mybir.ActivationFunctionType.Exp, scale=SCALE)
                        if masked:
                            nc.vector.tensor_tensor(
                                es[:], es[:],
                                mask_sb[:, kc, None, :].to_broadcast((128, 2, R)),
                                mybir.AluOpType.mult)
                        nc.tensor.matmul(psA[:], lhsT=vts[0][:, kc, :],
                                         rhs=es[:, 0, :],
                                         start=(kc == 0), stop=(kc == KC - 1))
                        nc.tensor.matmul(psB[:], lhsT=vts[1][:, kc, :],
                                         rhs=es[:, 1, :],
                                         start=(kc == 0), stop=(kc == KC - 1))
                    # normalize: rows 0..63 / row 64
                    for hh, pso in ((0, psA), (1, psB)):
                        # denom lives at psum partition 64; ACT copies it to
                        # SBUF (lane-aligned), a K=1 matmul broadcasts the RAW
                        # denominator to partitions 0..63, then one wide DVE
                        # reciprocal produces 1/denom on all 64 lanes at once.
                        rec = pool.tile([HD + 1, R], F32, tag="rec",
                                        name=f"r_{name}_{hp}_{hh}")
                        nc.vector.tensor_copy(rec[HD:HD + 1, :],
                                              pso[HD:HD + 1, :])
                        pbig = psum_pool.tile([128, 2 * R], F32, tag="psS",
                                              bufs=3, name=f"b_{name}_{hp}_{hh}")
                        pbc = pbig[0:HD, 0:R]
                        nc.tensor.matmul(pbc,
                                         lhsT=ones_full[HD:HD + 1, 0:HD],
                                         rhs=rec[HD:HD + 1, :],
                                         start=True, stop=True)
                        bcs = pool.tile([HD, R], F32, tag="bcs",
                                        name=f"c_{name}_{hp}_{hh}")
                        nc.vector.reciprocal_approx_fast(bcs[:], pbc)
                        if hh == 0:
                            nc.vector.tensor_tensor(
                                dst_sb[0:HD, hp, :], pso[0:HD, :], bcs[:],
                                mybir.AluOpType.mult)
                        else:
                            tmb = pool.tile([HD, R], BF16, tag="tmb",
                                            name=f"t_{name}_{hp}")
                            nc.vector.tensor_tensor(tmb[:], pso[0:HD, :], bcs[:],
                                                    mybir.AluOpType.mult)
                            nc.sync.dma_start(dst_sb[HD:128, hp, :], tmb[:])

            # ================= phase 1: ln1, qkv, cross KV, AllGathers ========
            with (
                tc.tile_pool(name="p1", bufs=2) as pool,
                tc.tile_pool(name="p1ps", bufs=2, space="PSUM") as psum_pool,
            ):
                ln1 = layernorm(x_sb, pool, psum_pool, "ln1")

                def eat_k(oc, ps):
                    kl = pool.tile([128, R], BF16, tag="kvcopy", name=f"kl_{oc}")
                    nc.vector.tensor_scalar_add(kl[:], ps[:], bias_ap(B_K, oc))
                    half, ocl = oc // (DC // 2), oc % (DC // 2)
                    nc.sync.dma_start(
                        ktS_in[half][ocl * 128:(ocl + 1) * 128, :], kl[:])

                def eat_vs(rc, ps):
                    vl = pool.tile([128, D], BF16, tag="vcopy", name=f"vl_{rc}")
                    nc.vector.tensor_copy(vl[:], ps[:])
                    for half in range(2):
                        nc.sync.dma_start(
                            vS_in[half][:, rc * 128:(rc + 1) * 128, :]
                            .rearrange("h p d -> p h d"),
                            vl[:, half * DH:(half + 1) * DH]
                            .rearrange("p (h d) -> p h d", d=HD))

                # half A of self K, then V, so the first AllGather can launch
                # while the rest of phase 1 computes
                matmul_t(ln1, wk, DC, DC, pool, psum_pool, "k", eat_k,
                         ocs=range(0, DC // 2))
                matmul_normal_v(ln1, wv, bv_sb, pool, psum_pool, "vs", eat_vs)
                allgather(ktS_in[0], ktS_out[0])
                allgather(vS_in[0], vS_out[0])
                matmul_t(ln1, wk, DC, DC, pool, psum_pool, "k2", eat_k,
                         ocs=range(DC // 2, DC))
                allgather(ktS_in[1], ktS_out[1])
                allgather(vS_in[1], vS_out[1])

                def eat_q(oc, ps):
                    nc.scalar.activation(qt_sb[:, oc, :], ps[:],
                                         mybir.ActivationFunctionType.Identity,
                                         bias=bias_ap(B_Q, oc))

                matmul_t(ln1, wq, DC, DC, pool, psum_pool, "q", eat_q)

                # cross K/V from enc_out (no LN)
                def eat_kc(oc, ps):
                    kl = pool.tile([128, R], BF16, tag="kvcopy", name=f"kc_{oc}")
                    nc.vector.tensor_scalar_add(kl[:], ps[:], bias_ap(B_KVK, oc))
                    half, ocl = oc // (DC // 2), oc % (DC // 2)
                    nc.sync.dma_start(
                        ktC_in[half][ocl * 128:(ocl + 1) * 128, :], kl[:])

                def eat_vc(rc, ps):
                    vl = pool.tile([128, D], BF16, tag="vcopy", name=f"vc_{rc}")
                    nc.vector.tensor_copy(vl[:], ps[:])
                    for half in range(2):
                        nc.sync.dma_start(
                            vC_in[half][:, rc * 128:(rc + 1) * 128, :]
                            .rearrange("h p d -> p h d"),
                            vl[:, half * DH:(half + 1) * DH]
                            .rearrange("p (h d) -> p h d", d=HD))

                matmul_t(enc_sb, wkvk, DC, DC, pool, psum_pool, "kc", eat_kc)
                matmul_normal_v(enc_sb, wkvv, bkvv_sb, pool, psum_pool, "vc", eat_vc)
                allgather(ktC_in[0], ktC_out[0])
                allgather(vC_in[0], vC_out[0])
                allgather(ktC_in[1], ktC_out[1])
                allgather(vC_in[1], vC_out[1])

            # ================= phase 2: self attention ========================
            with (
                tc.tile_pool(name="p2", bufs=2) as pool,
                tc.tile_pool(name="p2ps", bufs=2, space="PSUM") as psum_pool,
            ):
                attention(qt_sb, ktS_out, vS_out, at_self, True, pool, psum_pool,
                          "sa")

            # ================= phase 3: proj + residual, ln2, q2 ==============
            with (
                tc.tile_pool(name="p3", bufs=2) as pool,
                tc.tile_pool(name="p3ps", bufs=2, space="PSUM") as psum_pool,
            ):
                def eat_proj(oc, ps):
                    nc.vector.scalar_tensor_tensor(
                        x2_sb[:, oc, :], ps[:], bias_ap(B_PROJ, oc),
                        x_sb[:, oc, :],
                        mybir.AluOpType.add, mybir.AluOpType.add)

                matmul_t(at_self, wproj, DC, DC, pool, psum_pool, "pr", eat_proj)

                ln2 = layernorm(x2_sb, pool, psum_pool, "ln2")

                def eat_q2(oc, ps):
                    nc.scalar.activation(qt2_sb[:, oc, :], ps[:],
                                         mybir.ActivationFunctionType.Identity,
                                         bias=bias_ap(B_Q2, oc))

                matmul_t(ln2, wq2, DC, DC, pool, psum_pool, "q2", eat_q2)

            # ================= phase 4: cross attention =======================
            with (
                tc.tile_pool(name="p4", bufs=2) as pool,
                tc.tile_pool(name="p4ps", bufs=2, space="PSUM") as psum_pool,
            ):
                attention(qt2_sb, ktC_out, vC_out, at_cross, False, pool,
                          psum_pool, "ca")

            # ================= phase 5: co + residual, ln3, MLP ===============
            with (
                tc.tile_pool(name="p5", bufs=2) as pool,
                tc.tile_pool(name="p5ps", bufs=2, space="PSUM") as psum_pool,
            ):
                def eat_co(oc, ps):
                    nc.vector.scalar_tensor_tensor(
                        x3_sb[:, oc, :], ps[:], bias_ap(B_CO, oc),
                        x2_sb[:, oc, :],
                        mybir.AluOpType.add, mybir.AluOpType.add)

                matmul_t(at_cross, wco, DC, DC, pool, psum_pool, "co", eat_co)

                ln3 = layernorm(x3_sb, pool, psum_pool, "ln3")

                h_sb = pool.tile([128, MC, R], BF16, tag="hsb", bufs=1)

                def eat_m1(oc, ps):
                    nc.scalar.activation(h_sb[:, oc, :], ps[:],
                                         mybir.ActivationFunctionType.Gelu,
                                         bias=bias_ap(B_M1, oc))

                matmul_t(ln3, wm1, DC, MC, pool, psum_pool, "m1", eat_m1)

                # x_sb is dead by now — reuse it as the output buffer
                def eat_m2(oc, ps):
                    nc.vector.scalar_tensor_tensor(
                        x_sb[:, oc, :], ps[:], bias_ap(B_M2, oc),
                        x3_sb[:, oc, :],
                        mybir.AluOpType.add, mybir.AluOpType.add)
                    nc.sync.dma_start(
                        out_t.ap().rearrange("(c p) f -> p c f", p=128)[:, oc, :],
                        x_sb[:, oc, :])

                matmul_t(h_sb, wm2, MC, DC, pool, psum_pool, "m2", eat_m2,
                         w_tag="wtile2")

    nc.finalize()
    return nc


def prep_inputs(inputs):
    """Host-side prep: fold LN affine into weights, cast/tile, shard rows."""
    f32 = np.float32
    bf16 = ml_dtypes.bfloat16

    def tile_w(w, nk, no):
        # [nk*128, no*128] -> [nk, no, 128, 128]
        return np.ascontiguousarray(
            w.reshape(nk, 128, no, 128).transpose(0, 2, 1, 3)).astype(bf16)

    def chunk_b(b, n):
        return np.ascontiguousarray(b.reshape(n, 128)).astype(f32)

    x = np.asarray(inputs["x"], f32)
    enc = np.asarray(inputs["enc_out"], f32)
    cm = np.asarray(inputs["causal_mask"])

    ln1_g, ln1_b = np.asarray(inputs["ln1_g"], f32), np.asarray(inputs["ln1_b"], f32)
    ln2_g, ln2_b = np.asarray(inputs["ln2_g"], f32), np.asarray(inputs["ln2_b"], f32)
    ln3_g, ln3_b = np.asarray(inputs["ln3_g"], f32), np.asarray(inputs["ln3_b"], f32)
    qkv_w = np.asarray(inputs["qkv_w"], f32)
    qkv_b = np.asarray(inputs["qkv_b"], f32)
    q_w, q_b = np.asarray(inputs["q_w"], f32), np.asarray(inputs["q_b"], f32)
    kv_w, kv_b = np.asarray(inputs["kv_w"], f32), np.asarray(inputs["kv_b"], f32)
    mlp1_w, mlp1_b = np.asarray(inputs["mlp1_w"], f32), np.asarray(inputs["mlp1_b"], f32)

    qkv_w_eff = ln1_g[:, None] * qkv_w
    qkv_b_eff = qkv_b + ln1_b @ qkv_w
    q_w_eff = ln2_g[:, None] * q_w
    q_b_eff = q_b + ln2_b @ q_w
    m1_w_eff = ln3_g[:, None] * mlp1_w
    m1_b_eff = mlp1_b + ln3_b @ mlp1_w

    shared = {
        "wq": tile_w(qkv_w_eff[:, 0:D], DC, DC),
        "wk": tile_w(qkv_w_eff[:, D:2 * D], DC, DC),
        "wv": np.ascontiguousarray(
            qkv_w_eff[:, 2 * D:3 * D].reshape(DC, 128, D)).astype(bf16),
        "wproj": tile_w(np.asarray(inputs["proj_w"], f32), DC, DC),
        "wq2": tile_w(q_w_eff, DC, DC),
        "wkvk": tile_w(kv_w[:, 0:D], DC, DC),
        "wkvv": np.ascontiguousarray(
            kv_w[:, D:2 * D].reshape(DC, 128, D)).astype(bf16),
        "wco": tile_w(np.asarray(inputs["co_w"], f32), DC, DC),
        "wm1": tile_w(m1_w_eff, DC, MC),
        "wm2": tile_w(np.asarray(inputs["mlp2_w"], f32), MC, DC),
        "bq": chunk_b(qkv_b_eff[0:D], DC),
        "bk": chunk_b(qkv_b_eff[D:2 * D], DC),
        "bv": qkv_b_eff[2 * D:3 * D].reshape(1, D).astype(bf16),
        "bproj": chunk_b(np.asarray(inputs["proj_b"], f32), DC),
        "bq2": chunk_b(q_b_eff, DC),
        "bkvk": chunk_b(kv_b[0:D], DC),
        "bkvv": kv_b[D:2 * D].reshape(1, D).astype(bf16),
        "bco": chunk_b(np.asarray(inputs["co_b"], f32), DC),
        "bm1": chunk_b(m1_b_eff, MC),
        "bm2": chunk_b(np.asarray(inputs["mlp2_b"], f32), DC),
    }

    in_maps = []
    for c in range(N_CORES):
        b = c // GROUP
        r0 = (c % GROUP) * R
        m = dict(shared)
        m["x_t"] = np.ascontiguousarray(x[b, r0:r0 + R].T)
        m["enc_tb"] = np.ascontiguousarray(enc[b, r0:r0 + R].T).astype(bf16)
        m["mask_t"] = np.ascontiguousarray(
            (cm != 0).astype(f32)[r0:r0 + R].T).astype(bf16)
        in_maps.append(m)
    return in_maps


_prog_cache = {}


def kernel(**inputs):
    if "nc" not in _prog_cache:
        _prog_cache["nc"] = build_program()
    nc = _prog_cache["nc"]
    in_maps = prep_inputs(inputs)
    res = run_bass_kernel_spmd(nc, in_maps, core_ids=list(range(N_CORES)))
    out = np.empty((B, T, D), np.float32)
    for c in range(N_CORES):
        b = c // GROUP
        r0 = (c % GROUP) * R
        out[b, r0:r0 + R] = res.results[c]["out_t"].T
    _prog_cache["last_results"] = res
    return out



# revision 37
# speedup vs baseline: 1.1001x; 1.1001x over previous
"""Trainium2 Bass kernel for nn_DecoderBlock (B=2, T=2048, D=1024, H=16, MLP=4096).

Sharding: sequence/row parallel over 8 cores (4 cores per batch, 512 rows each).
K/V for both attentions are computed on local rows and AllGathered (2 groups of
4 cores).  K and V are packed into ONE gather buffer per head-half (4 gathers
total), launched as early as possible so they overlap Q / cross-KV compute.

On-device layout: activations transposed [feature, row]; LayerNorm affine
params folded into the following weight matrix on the host.  Weights are
pre-tiled on the host so every SBUF weight tile is one contiguous DRAM block
(fat DMA descriptors).  Softmax without max-subtraction; mask applied as 0/1
multiply on exp(scores); denominator comes from a ones column interleaved with
V in the gather buffer ([r, 8*65] layout).  LN rstd via exp(-0.5*ln(var+eps))
so the whole kernel uses one ACT table set until the final Gelu.
"""

import sys

if "/opt/trn_rl_repo" not in sys.path:
    sys.path.insert(0, "/opt/trn_rl_repo")

import numpy as np
import ml_dtypes

import concourse.bass as bass
import concourse.mybir as mybir
import concourse.tile as tile
from concourse import bacc
from concourse.bass_utils import run_bass_kernel_spmd

F32 = mybir.dt.float32
BF16 = mybir.dt.bfloat16

B, T, D, H, HD = 2, 2048, 1024, 16, 64
MLP = 4 * D
EPS = 1e-5
N_CORES = 8
GROUP = 4            # cores per batch element
R = T // GROUP       # rows per core = 512
DC = D // 128        # feature chunks = 8
MC = MLP // 128      # mlp chunks = 32
KC = T // 128        # key chunks = 16
NPAIR = H // 2       # head pairs = 8
SCALE = HD ** -0.5
VW = 8 * (HD + 1)    # V gather row width (ones interleaved) = 520
GW = 512 + VW        # gather buffer row width = 1032
NB = 256             # row block (2 zigzag blocks per core)


def build_program(dbg=False):
    nc = bacc.Bacc("TRN2", target_bir_lowering=False, debug=False,
                   num_devices=N_CORES)
    dbg_t = {}
    if dbg:
        dbg_t["ln1"] = nc.dram_tensor("d_ln1", [128, DC, R], BF16,
                                      kind="ExternalOutput")
        dbg_t["qt"] = nc.dram_tensor("d_qt", [128, DC, R], BF16,
                                     kind="ExternalOutput")
        dbg_t["at"] = nc.dram_tensor("d_at", [128, DC, R], BF16,
                                     kind="ExternalOutput")
        dbg_t["x2"] = nc.dram_tensor("d_x2", [128, DC, R], F32,
                                     kind="ExternalOutput")
        dbg_t["pss0"] = nc.dram_tensor("d_pss0", [128, 2, R], F32,
                                       kind="ExternalOutput")
        dbg_t["es0"] = nc.dram_tensor("d_es0", [128, 2, R], BF16,
                                      kind="ExternalOutput")
        dbg_t["psA"] = nc.dram_tensor("d_psA", [HD + 1, R], F32,
                                      kind="ExternalOutput")
        dbg_t["psB"] = nc.dram_tensor("d_psB", [HD + 1, R], F32,
                                      kind="ExternalOutput")
        dbg_t["bcs"] = nc.dram_tensor("d_bcs", [HD, 2, R], F32,
                                      kind="ExternalOutput")

    # ---- DRAM I/O ----
    x_t = nc.dram_tensor("x_t", [128, DC, R], F32, kind="ExternalInput")
    enc_t = nc.dram_tensor("enc_t", [128, DC, R], BF16, kind="ExternalInput")
    mask_t = nc.dram_tensor("mask_t", [128, KC, R], BF16, kind="ExternalInput")
    # dense weights pre-tiled [n_out, 128, n_k, 128]
    wq = nc.dram_tensor("wq", [DC, 128, DC, 128], BF16, kind="ExternalInput")
    wk = nc.dram_tensor("wk", [DC, 128, DC, 128], BF16, kind="ExternalInput")
    wv = nc.dram_tensor("wv", [2, DC, 128, 512], BF16, kind="ExternalInput")
    wproj = nc.dram_tensor("wproj", [DC, 128, DC, 128], BF16, kind="ExternalInput")
    wq2 = nc.dram_tensor("wq2", [DC, 128, DC, 128], BF16, kind="ExternalInput")
    wkvk = nc.dram_tensor("wkvk", [DC, 128, DC, 128], BF16, kind="ExternalInput")
    wkvv = nc.dram_tensor("wkvv", [2, DC, 128, 512], BF16, kind="ExternalInput")
    wco = nc.dram_tensor("wco", [DC, 128, DC, 128], BF16, kind="ExternalInput")
    wm1 = nc.dram_tensor("wm1", [MC, 128, DC, 128], BF16, kind="ExternalInput")
    wm2 = nc.dram_tensor("wm2", [DC, 128, MC, 128], BF16, kind="ExternalInput")
    bq = nc.dram_tensor("bq", [DC, 128], F32, kind="ExternalInput")
    bk = nc.dram_tensor("bk", [DC, 128], F32, kind="ExternalInput")
    bv = nc.dram_tensor("bv", [1, D], BF16, kind="ExternalInput")
    bproj = nc.dram_tensor("bproj", [DC, 128], F32, kind="ExternalInput")
    bq2 = nc.dram_tensor("bq2", [DC, 128], F32, kind="ExternalInput")
    bkvk = nc.dram_tensor("bkvk", [DC, 128], F32, kind="ExternalInput")
    bkvv = nc.dram_tensor("bkvv", [1, D], BF16, kind="ExternalInput")
    bco = nc.dram_tensor("bco", [DC, 128], F32, kind="ExternalInput")
    bm1 = nc.dram_tensor("bm1", [MC, 128], F32, kind="ExternalInput")
    bm2 = nc.dram_tensor("bm2", [DC, 128], F32, kind="ExternalInput")
    out_t = nc.dram_tensor("out_t", [128, DC, R], F32, kind="ExternalOutput")

    rg = [[0, 1, 2, 3], [4, 5, 6, 7]]
    AFT = mybir.ActivationFunctionType

    with tile.TileContext(nc) as tc:
        with (
            tc.tile_pool(name="persist", bufs=1) as pp,
            tc.tile_pool(name="dram", bufs=1, space="DRAM") as dram,
        ):
            # gather buffers (<=520KB so collectives take the mesh path):
            # K^T half [512 feat, 512 tok]; V half [512 tok, 8*65] ones-interleaved
            kin_s = [dram.tile([512, 512], BF16, name=f"kinS{i}") for i in range(2)]
            kout_s = [dram.tile([2048, 512], BF16, name=f"koutS{i}") for i in range(2)]
            vin_s = [dram.tile([512, VW], BF16, name=f"vinS{i}") for i in range(2)]
            vout_s = [dram.tile([2048, VW], BF16, name=f"voutS{i}") for i in range(2)]
            kin_c = [dram.tile([512, 512], BF16, name=f"kinC{i}") for i in range(2)]
            kout_c = [dram.tile([2048, 512], BF16, name=f"koutC{i}") for i in range(2)]
            vin_c = [dram.tile([512, VW], BF16, name=f"vinC{i}") for i in range(2)]
            vout_c = [dram.tile([2048, VW], BF16, name=f"voutC{i}") for i in range(2)]

            def allgather(src_d, dst_d):
                nc.gpsimd.collective_compute(
                    "AllGather", mybir.AluOpType.bypass,
                    ins=[src_d.opt()], outs=[dst_d.opt()], replica_groups=rg)

            # persistent SBUF
            x_sb = pp.tile([128, DC, R], F32)        # residual stream x^T
            x2_sb = pp.tile([128, DC, R], F32)
            enc_sb = pp.tile([128, DC, R], BF16)
            mask_sb = pp.tile([128, KC, R], BF16)
            ln1_sb = pp.tile([128, DC, R], BF16)
            qt_sb = pp.tile([128, DC, R], BF16)      # Q^T (self)
            qt2_sb = pp.tile([128, DC, R], BF16)     # Q^T (cross)
            at_sb = pp.tile([128, DC, R], BF16)      # attn out^T (reused)
            ones_rbf = pp.tile([1, 128], BF16)       # K=1 lhsT for bf16 bcasts
            ones_f32 = pp.tile([128, 128], F32)      # K=1 lhsT rows for f32 bcasts
            ones_bf_col = pp.tile([128, 1], BF16)    # lhsT for partition sums
            bias_sb = pp.tile([128, 8 * DC + MC], F32)
            bv_sb = pp.tile([1, D], BF16)
            bkvv_sb = pp.tile([1, D], BF16)
            eps_sb = pp.tile([1, 1], F32)

            nc.vector.memset(ones_rbf[:], 1.0)
            nc.vector.memset(ones_f32[:], 1.0)
            nc.vector.memset(ones_bf_col[:], 1.0)
            nc.vector.memset(eps_sb[:], EPS)

            for c in range(DC):
                nc.sync.dma_start(x_sb[:, c, :], x_t.ap()[:, c, :])
                nc.sync.dma_start(enc_sb[:, c, :], enc_t.ap()[:, c, :])
            nc.sync.dma_start(bv_sb[:], bv.ap())
            nc.sync.dma_start(bkvv_sb[:], bkvv.ap())
            bias_list = [bq, bk, bproj, bq2, bkvk, bco, bm2]
            for i, b in enumerate(bias_list):
                nc.sync.dma_start(
                    bias_sb[:, i * DC:(i + 1) * DC],
                    b.ap().rearrange("c p -> p c"),
                )
            nc.sync.dma_start(
                bias_sb[:, 7 * DC:7 * DC + MC], bm1.ap().rearrange("c p -> p c")
            )
            B_Q, B_K, B_PROJ, B_Q2, B_KVK, B_CO, B_M2 = (
                0, DC, 2 * DC, 3 * DC, 4 * DC, 5 * DC, 6 * DC)
            B_M1 = 7 * DC

            def bias_ap(base, oc):
                return bias_sb[:, base + oc:base + oc + 1]

            # ---------- helpers ----------
            def layernorm(src_sb, dst_sb, pool, psum_pool, name):
                """src_sb [128, DC, R] f32 -> ln^T bf16 into dst_sb (no affine)."""
                ps1 = psum_pool.tile([1, R], F32, tag="stats", bufs=2,
                                     name=f"p1_{name}")
                ps2 = psum_pool.tile([1, R], F32, tag="stats", bufs=2,
                                     name=f"p2_{name}")
                for c in range(DC):
                    xb = pool.tile([128, R], BF16, tag="lnxb", name=f"xb_{name}{c}")
                    sq = pool.tile([128, R], BF16, tag="lnsq", name=f"sq_{name}{c}")
                    nc.vector.tensor_copy(xb[:], src_sb[:, c, :])
                    nc.vector.tensor_tensor(sq[:], xb[:], xb[:],
                                            mybir.AluOpType.mult)
                    nc.tensor.matmul(ps1[:], lhsT=ones_bf_col[:], rhs=xb[:],
                                     start=(c == 0), stop=(c == DC - 1))
                    nc.tensor.matmul(ps2[:], lhsT=ones_bf_col[:], rhs=sq[:],
                                     start=(c == 0), stop=(c == DC - 1))
                nmean = pool.tile([1, R], F32, tag="lnrow", bufs=8, name=f"nm_{name}")
                ex2 = pool.tile([1, R], F32, tag="lnrow", bufs=8, name=f"e2_{name}")
                m2 = pool.tile([1, R], F32, tag="lnrow", bufs=8, name=f"m2_{name}")
                var = pool.tile([1, R], F32, tag="lnrow", bufs=8, name=f"va_{name}")
                lnv = pool.tile([1, R], F32, tag="lnrow", bufs=8, name=f"lv_{name}")
                rstd = pool.tile([1, R], F32, tag="lnrow", bufs=8, name=f"rs_{name}")
                nmrs = pool.tile([1, R], F32, tag="lnrow", bufs=8, name=f"nr_{name}")
                nc.scalar.activation(nmean[:], ps1[:], AFT.Identity,
                                     scale=-1.0 / D)
                nc.scalar.activation(ex2[:], ps2[:], AFT.Identity, scale=1.0 / D)
                nc.vector.tensor_tensor(m2[:], nmean[:], nmean[:],
                                        mybir.AluOpType.mult)
                nc.vector.tensor_tensor(var[:], ex2[:], m2[:],
                                        mybir.AluOpType.subtract)
                # rstd = exp(-0.5 * ln(var + eps)) — stays in the exp/ln set
                nc.scalar.activation(lnv[:], var[:], AFT.Ln, bias=eps_sb[:])
                nc.scalar.activation(rstd[:], lnv[:], AFT.Exp, scale=-0.5)
                nc.vector.tensor_tensor(nmrs[:], nmean[:], rstd[:],
                                        mybir.AluOpType.mult)
                psb = psum_pool.tile([128, 2, R], F32, tag="lnb", bufs=1,
                                     name=f"pb_{name}")
                nc.tensor.matmul(psb[:, 0, :], lhsT=ones_f32[0:1, :],
                                 rhs=rstd[:], start=True, stop=True)
                nc.tensor.matmul(psb[:, 1, :], lhsT=ones_f32[0:1, :],
                                 rhs=nmrs[:], start=True, stop=True)
                tmp = pool.tile([128, R], F32, tag="lntmp", name=f"tp_{name}")
                for c in range(DC):
                    nc.vector.tensor_tensor(tmp[:], src_sb[:, c, :], psb[:, 0, :],
                                            mybir.AluOpType.mult)
                    nc.vector.tensor_tensor(dst_sb[:, c, :], tmp[:], psb[:, 1, :],
                                            mybir.AluOpType.add)

            def matmul_t(rhs_sb, w_dram, n_k, n_o, pool, psum_pool, name,
                         consume, w_tag="wtile", ocs=None, ps_bufs=3):
                """out^T[oc] = sum_kc W[oc][kc].T @ rhs[kc]; consume(oc, psum)."""
                for oc in (range(n_o) if ocs is None else ocs):
                    wt = pool.tile([128, n_k, 128], BF16, tag=w_tag,
                                   name=f"w_{name}_{oc}")
                    nc.gpsimd.dma_start(wt[:], w_dram.ap()[oc])
                    ps = psum_pool.tile([128, R], F32, tag="mm", bufs=ps_bufs,
                                        name=f"ps_{name}_{oc}")
                    for kc in range(n_k):
                        nc.tensor.matmul(ps[:], lhsT=wt[:, kc, :],
                                         rhs=rhs_sb[:, kc, :],
                                         start=(kc == 0), stop=(kc == n_k - 1))
                    consume(oc, ps)

            def kv_half(src_sb, wk_d, wv_d, bk_base, bv_row, kin, kout, vin,
                        vout, pool, psum_pool, half, name):
                """K^T half + V half (with ones cols) -> gather buffers + AGs."""
                # K^T: 4 output chunks = features [512*half, 512*half+512)
                def eat_k(oc, ps):
                    ocl = oc - half * (DC // 2)
                    kl = pool.tile([128, R], BF16, tag="kvcopy", name=f"kl_{name}{oc}")
                    nc.vector.tensor_scalar_add(kl[:], ps[:], bias_ap(bk_base, oc))
                    nc.sync.dma_start(
                        kin[ocl * 128:(ocl + 1) * 128, :], kl[:])

                matmul_t(src_sb, wk_d, DC, DC, pool, psum_pool, f"k{name}",
                         eat_k, ocs=range(half * (DC // 2), (half + 1) * (DC // 2)),
                         ps_bufs=4)
                allgather(kin, kout)

                # V: rows x 512 features of this half, + interleaved ones.
                # Weight tile loaded once per kc; 4 row-chunk psums accumulate.
                psvs = [psum_pool.tile([128, 512], F32, tag="mm", bufs=4,
                                       name=f"psv_{name}_{rc}")
                        for rc in range(R // 128)]
                for kc in range(DC):
                    wt = pool.tile([128, 512], BF16, tag="wv",
                                   name=f"wv_{name}_{kc}")
                    nc.gpsimd.dma_start(wt[:], wv_d.ap()[half, kc])
                    for rc in range(R // 128):
                        nc.tensor.matmul(
                            psvs[rc][:],
                            lhsT=src_sb[:, kc, rc * 128:(rc + 1) * 128],
                            rhs=wt[:], start=(kc == 0), stop=False)
                for rc in range(R // 128):
                    nc.tensor.matmul(psvs[rc][:], lhsT=ones_rbf[:],
                                     rhs=bv_row[:, half * 512:(half + 1) * 512],
                                     start=False, stop=True)
                    vl = pool.tile([128, 8, HD + 1], BF16, tag="vcopy",
                                   name=f"vl_{name}_{rc}")
                    nc.vector.tensor_copy(
                        vl[:, :, 0:HD],
                        psvs[rc][:].rearrange("p (h d) -> p h d", d=HD))
                    nc.vector.memset(vl[:, :, HD:HD + 1], 1.0)
                    nc.sync.dma_start(
                        vin[rc * 128:(rc + 1) * 128, :],
                        vl[:].rearrange("p h d -> p (h d)"))
                allgather(vin, vout)

            def attention(qt, kouts, vouts, dst_sb, masked, pool, psum_pool,
                          name):
                """dst_sb [128, DC, R] bf16 = attn(Q^T, gathered K/V)^T."""
                for hp in range(NPAIR):
                    half, hpl = hp // (NPAIR // 2), hp % (NPAIR // 2)
                    ko, vo = kouts[half], vouts[half]
                    # K^T tile: [feat 128, kc, key 128]
                    ktp = pool.tile([128, KC, 128], BF16, tag="ktp",
                                    name=f"kt_{name}_{hp}")
                    vt = pool.tile([128, KC, 2 * (HD + 1)], BF16, tag="vt",
                                   name=f"v_{name}_{hp}")
                    j0 = hpl * 2 * (HD + 1)
                    for s in range(4):
                        f0 = 512 * s + 128 * hpl
                        nc.sync.dma_start(
                            ktp[:, 4 * s:4 * s + 4, :],
                            ko[f0:f0 + 128, :]
                            .rearrange("p (c m) -> p c m", m=128))
                        # V tile: [key 128, kc, 2*(HD+1)] heads + ones cols
                        nc.sync.dma_start(
                            vt[:, 4 * s:4 * s + 4, :],
                            vo[512 * s:512 * s + 512, j0:j0 + 2 * (HD + 1)]
                            .rearrange("(c p) w -> p c w", p=128))
                    psA = psum_pool.tile([HD + 1, R], F32, tag="psO", bufs=2,
                                         name=f"oA_{name}_{hp}")
                    psB = psum_pool.tile([HD + 1, R], F32, tag="psO", bufs=2,
                                         name=f"oB_{name}_{hp}")
                    for kc in range(KC):
                        pss = psum_pool.tile([128, 2, R], F32, tag="psS", bufs=3,
                                             name=f"s_{name}_{hp}_{kc}")
                        nc.tensor.matmul(pss[:, 0, :],
                                         lhsT=ktp[0:64, kc, :],
                                         rhs=qt[0:64, hp, :],
                                         start=True, stop=True)
                        nc.tensor.matmul(pss[:, 1, :],
                                         lhsT=ktp[64:128, kc, :],
                                         rhs=qt[64:128, hp, :],
                                         start=True, stop=True)
                        if dbg and name == "sa" and hp == 0 and kc == 0:
                            pc = pool.tile([128, 2, R], F32, name="d_pssc")
                            nc.vector.tensor_copy(pc[:], pss[:])
                            nc.sync.dma_start(dbg_t["pss0"].ap(), pc[:])
                        es = pool.tile([128, 2, R], BF16, tag="expS", bufs=3,
                                       name=f"e_{name}_{hp}_{kc}")
                        nc.scalar.activation(
                            es[:].rearrange("p a f -> p (a f)"),
                            pss[:].rearrange("p a f -> p (a f)"),
                            AFT.Exp, scale=SCALE)
                        if masked:
                            nc.vector.tensor_tensor(
                                es[:], es[:],
                                mask_sb[:, kc, None, :].to_broadcast((128, 2, R)),
                                mybir.AluOpType.mult)
                        if dbg and name == "sa" and hp == 0 and kc == 0:
                            nc.sync.dma_start(dbg_t["es0"].ap(), es[:])
                        nc.tensor.matmul(psA[:], lhsT=vt[:, kc, 0:HD + 1],
                                         rhs=es[:, 0, :],
                                         start=(kc == 0), stop=(kc == KC - 1))
                        nc.tensor.matmul(psB[:], lhsT=vt[:, kc, HD + 1:],
                                         rhs=es[:, 1, :],
                                         start=(kc == 0), stop=(kc == KC - 1))
                    if dbg and name == "sa" and hp == 0:
                        pa = pool.tile([HD + 1, R], F32, name="d_psac")
                        pb = pool.tile([HD + 1, R], F32, name="d_psbc")
                        nc.vector.tensor_copy(pa[:], psA[:])
                        nc.vector.tensor_copy(pb[:], psB[:])
                        nc.sync.dma_start(dbg_t["psA"].ap(), pa[:])
                        nc.sync.dma_start(dbg_t["psB"].ap(), pb[:])
                    # normalize: copy raw denom, PE-broadcast to 64 lanes,
                    # reciprocal, multiply (baseline-proven pattern)
                    rec = pool.tile([HD + 1, R], F32, tag="rec", bufs=4,
                                    name=f"r_{name}_{hp}")
                    nc.vector.tensor_copy(rec[HD:HD + 1, 0:R],
                                          psA[HD:HD + 1, :])
                    recB = pool.tile([HD + 1, R], F32, tag="recB", bufs=4,
                                     name=f"rB_{name}_{hp}")
                    nc.vector.tensor_copy(recB[HD:HD + 1, 0:R],
                                          psB[HD:HD + 1, :])
                    psn = psum_pool.tile([128, 2, R], F32, tag="psS", bufs=3,
                                         name=f"n_{name}_{hp}")
                    nc.tensor.matmul(psn[0:HD, 0, :],
                                     lhsT=ones_f32[HD:HD + 1, 0:HD],
                                     rhs=rec[HD:HD + 1, :],
                                     start=True, stop=True)
                    nc.tensor.matmul(psn[0:HD, 1, :],
                                     lhsT=ones_f32[HD:HD + 1, 0:HD],
                                     rhs=recB[HD:HD + 1, :],
                                     start=True, stop=True)
                    bcs = pool.tile([HD, 2, R], F32, tag="bcs",
                                    name=f"c_{name}_{hp}")
                    nc.vector.reciprocal_approx_fast(bcs[:, 0, :],
                                                     psn[0:HD, 0, :])
                    nc.vector.reciprocal_approx_fast(bcs[:, 1, :],
                                                     psn[0:HD, 1, :])
                    if dbg and name == "sa" and hp == 0:
                        nc.sync.dma_start(dbg_t["bcs"].ap(), bcs[:])
                    nc.vector.tensor_tensor(
                        dst_sb[0:HD, hp, :], psA[0:HD, :], bcs[:, 0, :],
                        mybir.AluOpType.mult)
                    tmb = pool.tile([HD, R], BF16, tag="tmb",
                                    name=f"t_{name}_{hp}")
                    nc.vector.tensor_tensor(tmb[:], psB[0:HD, :],
                                            bcs[:, 1, :],
                                            mybir.AluOpType.mult)
                    nc.sync.dma_start(dst_sb[HD:128, hp, :], tmb[:])

            # ============ phase 1: ln1, K/V(+gathers), Q, cross K/V ==========
            with (
                tc.tile_pool(name="p1", bufs=2) as pool,
                tc.tile_pool(name="p1ps", bufs=2, space="PSUM") as psum_pool,
            ):
                layernorm(x_sb, ln1_sb, pool, psum_pool, "ln1")
                kv_half(ln1_sb, wk, wv, B_K, bv_sb, kin_s[0], kout_s[0],
                        vin_s[0], vout_s[0], pool, psum_pool, 0, "sA")
                kv_half(ln1_sb, wk, wv, B_K, bv_sb, kin_s[1], kout_s[1],
                        vin_s[1], vout_s[1], pool, psum_pool, 1, "sB")

                def eat_q(oc, ps):
                    nc.scalar.activation(qt_sb[:, oc, :], ps[:], AFT.Identity,
                                         bias=bias_ap(B_Q, oc))

                matmul_t(ln1_sb, wq, DC, DC, pool, psum_pool, "q", eat_q,
                         ps_bufs=4)

                kv_half(enc_sb, wkvk, wkvv, B_KVK, bkvv_sb, kin_c[0],
                        kout_c[0], vin_c[0], vout_c[0], pool, psum_pool,
                        0, "cA")
                kv_half(enc_sb, wkvk, wkvv, B_KVK, bkvv_sb, kin_c[1],
                        kout_c[1], vin_c[1], vout_c[1], pool, psum_pool,
                        1, "cB")

            nc.sync.dma_start(mask_sb[:], mask_t.ap())
            if dbg:
                nc.sync.dma_start(dbg_t["ln1"].ap(), ln1_sb[:])
                nc.sync.dma_start(dbg_t["qt"].ap(), qt_sb[:])

            # ============ phase 2: self attention ============================
            with (
                tc.tile_pool(name="p2", bufs=2) as pool,
                tc.tile_pool(name="p2ps", bufs=2, space="PSUM") as psum_pool,
            ):
                attention(qt_sb, kout_s, vout_s, at_sb, True, pool, psum_pool,
                          "sa")
            if dbg:
                nc.sync.dma_start(dbg_t["at"].ap(), at_sb[:])

            # ============ phase 3: proj + residual, ln2, q2 ==================
            with (
                tc.tile_pool(name="p3", bufs=2) as pool,
                tc.tile_pool(name="p3ps", bufs=2, space="PSUM") as psum_pool,
            ):
                def eat_proj(oc, ps):
                    nc.vector.scalar_tensor_tensor(
                        x2_sb[:, oc, :], ps[:], bias_ap(B_PROJ, oc),
                        x_sb[:, oc, :],
                        mybir.AluOpType.add, mybir.AluOpType.add)

                matmul_t(at_sb, wproj, DC, DC, pool, psum_pool, "pr", eat_proj)
                if dbg:
                    nc.sync.dma_start(dbg_t["x2"].ap(), x2_sb[:])
                layernorm(x2_sb, ln1_sb, pool, psum_pool, "ln2")

                def eat_q2(oc, ps):
                    nc.scalar.activation(qt2_sb[:, oc, :], ps[:], AFT.Identity,
                                         bias=bias_ap(B_Q2, oc))

                matmul_t(ln1_sb, wq2, DC, DC, pool, psum_pool, "q2", eat_q2)

            # ============ phase 4: cross attention ===========================
            with (
                tc.tile_pool(name="p4", bufs=2) as pool,
                tc.tile_pool(name="p4ps", bufs=2, space="PSUM") as psum_pool,
            ):
                attention(qt2_sb, kout_c, vout_c, at_sb, False, pool,
                          psum_pool, "ca")

            # ============ phase 5: co + residual, ln3, MLP ===================
            with (
                tc.tile_pool(name="p5", bufs=2) as pool,
                tc.tile_pool(name="p5ps", bufs=2, space="PSUM") as psum_pool,
            ):
                def eat_co(oc, ps):
                    nc.vector.scalar_tensor_tensor(
                        x_sb[:, oc, :], ps[:], bias_ap(B_CO, oc),
                        x2_sb[:, oc, :],
                        mybir.AluOpType.add, mybir.AluOpType.add)

                matmul_t(at_sb, wco, DC, DC, pool, psum_pool, "co", eat_co)
                layernorm(x_sb, ln1_sb, pool, psum_pool, "ln3")

                h_sb = pool.tile([128, MC, R], BF16, tag="hsb", bufs=1)

                def eat_m1(oc, ps):
                    nc.scalar.activation(h_sb[:, oc, :], ps[:], AFT.Gelu,
                                         bias=bias_ap(B_M1, oc))

                matmul_t(ln1_sb, wm1, DC, MC, pool, psum_pool, "m1", eat_m1)

                def eat_m2(oc, ps):
                    nc.vector.scalar_tensor_tensor(
                        x2_sb[:, oc, :], ps[:], bias_ap(B_M2, oc),
                        x_sb[:, oc, :],
                        mybir.AluOpType.add, mybir.AluOpType.add)
                    nc.sync.dma_start(out_t.ap()[:, oc, :], x2_sb[:, oc, :])

                matmul_t(h_sb, wm2, MC, DC, pool, psum_pool, "m2", eat_m2,
                         w_tag="wtile2")

    nc.finalize()
    return nc


def prep_inputs(inputs):
    """Host-side prep: fold LN affine into weights, cast/tile, shard rows."""
    f32 = np.float32
    bf16 = ml_dtypes.bfloat16

    def tile_w(w, nk, no):
        # [nk*128, no*128] -> [no, 128, nk, 128] (contiguous per-oc tiles)
        return np.ascontiguousarray(
            w.reshape(nk, 128, no, 128).transpose(2, 1, 0, 3)).astype(bf16)

    def tile_v(w):
        # [D, D] -> [2, DC, 128, 512] (contiguous [128, 512] tiles per half)
        return np.ascontiguousarray(
            w.reshape(DC, 128, 2, 512).transpose(2, 0, 1, 3)).astype(bf16)

    def chunk_b(b, n):
        return np.ascontiguousarray(b.reshape(n, 128)).astype(f32)

    def chunk_t(a):
        # [rows, D] -> [128, DC, rows] transposed chunked
        return np.ascontiguousarray(
            a.T.reshape(DC, 128, -1).transpose(1, 0, 2))

    x = np.asarray(inputs["x"], f32)
    enc = np.asarray(inputs["enc_out"], f32)
    cm = np.asarray(inputs["causal_mask"])

    ln1_g, ln1_b = np.asarray(inputs["ln1_g"], f32), np.asarray(inputs["ln1_b"], f32)
    ln2_g, ln2_b = np.asarray(inputs["ln2_g"], f32), np.asarray(inputs["ln2_b"], f32)
    ln3_g, ln3_b = np.asarray(inputs["ln3_g"], f32), np.asarray(inputs["ln3_b"], f32)
    qkv_w = np.asarray(inputs["qkv_w"], f32)
    qkv_b = np.asarray(inputs["qkv_b"], f32)
    q_w, q_b = np.asarray(inputs["q_w"], f32), np.asarray(inputs["q_b"], f32)
    kv_w, kv_b = np.asarray(inputs["kv_w"], f32), np.asarray(inputs["kv_b"], f32)
    mlp1_w, mlp1_b = np.asarray(inputs["mlp1_w"], f32), np.asarray(inputs["mlp1_b"], f32)

    qkv_w_eff = ln1_g[:, None] * qkv_w
    qkv_b_eff = qkv_b + ln1_b @ qkv_w
    q_w_eff = ln2_g[:, None] * q_w
    q_b_eff = q_b + ln2_b @ q_w
    m1_w_eff = ln3_g[:, None] * mlp1_w
    m1_b_eff = mlp1_b + ln3_b @ mlp1_w

    shared = {
        "wq": tile_w(qkv_w_eff[:, 0:D], DC, DC),
        "wk": tile_w(qkv_w_eff[:, D:2 * D], DC, DC),
        "wv": tile_v(qkv_w_eff[:, 2 * D:3 * D]),
        "wproj": tile_w(np.asarray(inputs["proj_w"], f32), DC, DC),
        "wq2": tile_w(q_w_eff, DC, DC),
        "wkvk": tile_w(kv_w[:, 0:D], DC, DC),
        "wkvv": tile_v(kv_w[:, D:2 * D]),
        "wco": tile_w(np.asarray(inputs["co_w"], f32), DC, DC),
        "wm1": tile_w(m1_w_eff, DC, MC),
        "wm2": tile_w(np.asarray(inputs["mlp2_w"], f32), MC, DC),
        "bq": chunk_b(qkv_b_eff[0:D], DC),
        "bk": chunk_b(qkv_b_eff[D:2 * D], DC),
        "bv": qkv_b_eff[2 * D:3 * D].reshape(1, D).astype(bf16),
        "bproj": chunk_b(np.asarray(inputs["proj_b"], f32), DC),
        "bq2": chunk_b(q_b_eff, DC),
        "bkvk": chunk_b(kv_b[0:D], DC),
        "bkvv": kv_b[D:2 * D].reshape(1, D).astype(bf16),
        "bco": chunk_b(np.asarray(inputs["co_b"], f32), DC),
        "bm1": chunk_b(m1_b_eff, MC),
        "bm2": chunk_b(np.asarray(inputs["mlp2_b"], f32), DC),
    }

    mask_f = (cm != 0).astype(f32)
    in_maps = []
    for c in range(N_CORES):
        b = c // GROUP
        r0 = (c % GROUP) * R
        m = dict(shared)
        m["x_t"] = chunk_t(x[b, r0:r0 + R]).astype(f32)
        m["enc_t"] = chunk_t(enc[b, r0:r0 + R]).astype(bf16)
        m["mask_t"] = np.ascontiguousarray(
            mask_f[r0:r0 + R].T.reshape(KC, 128, R).transpose(1, 0, 2)
        ).astype(bf16)
        in_maps.append(m)
    return in_maps


_prog_cache = {}


def kernel(**inputs):
    if "nc" not in _prog_cache:
        _prog_cache["nc"] = build_program()
    nc = _prog_cache["nc"]
    in_maps = prep_inputs(inputs)
    res = run_bass_kernel_spmd(nc, in_maps, core_ids=list(range(N_CORES)))
    out = np.empty((B, T, D), np.float32)
    for c in range(N_CORES):
        b = c // GROUP
        r0 = (c % GROUP) * R
        # out_t [128, DC, R] -> [R, D]
        ot = res.results[c]["out_t"]
        out[b, r0:r0 + R] = ot.transpose(2, 1, 0).reshape(R, D)
    _prog_cache["last_results"] = res
    return out


# revision 48
# speedup vs baseline: 1.1129x; 1.0117x over previous
"""Trainium2 Bass kernel for nn_DecoderBlock (B=2, T=2048, D=1024, H=16, MLP=4096).

Sharding: sequence/row parallel over 8 cores (4 cores per batch, 512 rows each).
K/V for both attentions are computed on local rows and AllGathered (2 groups of
4 cores).  K and V are packed into ONE gather buffer per head-half (4 gathers
total), launched as early as possible so they overlap Q / cross-KV compute.

On-device layout: activations transposed [feature, row]; LayerNorm affine
params folded into the following weight matrix on the host.  Weights are
pre-tiled on the host so every SBUF weight tile is one contiguous DRAM block
(fat DMA descriptors).  Softmax without max-subtraction; mask applied as 0/1
multiply on exp(scores); denominator comes from a ones column interleaved with
V in the gather buffer ([r, 8*65] layout).  LN rstd via exp(-0.5*ln(var+eps))
so the whole kernel uses one ACT table set until the final Gelu.
"""

import sys

if "/opt/trn_rl_repo" not in sys.path:
    sys.path.insert(0, "/opt/trn_rl_repo")

import numpy as np
import ml_dtypes

import concourse.bass as bass
import concourse.mybir as mybir
import concourse.tile as tile
from concourse import bacc
from concourse.bass_utils import run_bass_kernel_spmd

F32 = mybir.dt.float32
BF16 = mybir.dt.bfloat16

B, T, D, H, HD = 2, 2048, 1024, 16, 64
MLP = 4 * D
EPS = 1e-5
N_CORES = 8
GROUP = 4            # cores per batch element
R = T // GROUP       # rows per core = 512
DC = D // 128        # feature chunks = 8
MC = MLP // 128      # mlp chunks = 32
KC = T // 128        # key chunks = 16
NPAIR = H // 2       # head pairs = 8
SCALE = HD ** -0.5
VW = 8 * (HD + 1)    # V gather row width (ones interleaved) = 520
GW = 512 + VW        # gather buffer row width = 1032
NB = 256             # row block (2 zigzag blocks per core)


def build_program(dbg=False):
    nc = bacc.Bacc("TRN2", target_bir_lowering=False, debug=False,
                   num_devices=N_CORES)
    dbg_t = {}
    if dbg:
        dbg_t["ln1"] = nc.dram_tensor("d_ln1", [128, DC, R], BF16,
                                      kind="ExternalOutput")
        dbg_t["qt"] = nc.dram_tensor("d_qt", [128, DC, R], BF16,
                                     kind="ExternalOutput")
        dbg_t["at"] = nc.dram_tensor("d_at", [128, DC, R], BF16,
                                     kind="ExternalOutput")
        dbg_t["x2"] = nc.dram_tensor("d_x2", [128, DC, R], F32,
                                     kind="ExternalOutput")
        dbg_t["pss0"] = nc.dram_tensor("d_pss0", [128, 2, R], F32,
                                       kind="ExternalOutput")
        dbg_t["es0"] = nc.dram_tensor("d_es0", [128, 2, R], BF16,
                                      kind="ExternalOutput")
        dbg_t["psA"] = nc.dram_tensor("d_psA", [HD + 1, R], F32,
                                      kind="ExternalOutput")
        dbg_t["psB"] = nc.dram_tensor("d_psB", [HD + 1, R], F32,
                                      kind="ExternalOutput")
        dbg_t["bcs"] = nc.dram_tensor("d_bcs", [HD, 2, R], F32,
                                      kind="ExternalOutput")

    # ---- DRAM I/O ----
    x_t = nc.dram_tensor("x_t", [128, DC, R], F32, kind="ExternalInput")
    enc_t = nc.dram_tensor("enc_t", [128, DC, R], BF16, kind="ExternalInput")
    mask_t = nc.dram_tensor("mask_t", [128, KC, R], BF16, kind="ExternalInput")
    # dense weights pre-tiled [n_out, 128, n_k, 128]
    wq = nc.dram_tensor("wq", [DC, 128, DC, 128], BF16, kind="ExternalInput")
    wk = nc.dram_tensor("wk", [DC, 128, DC, 128], BF16, kind="ExternalInput")
    wv = nc.dram_tensor("wv", [2, DC, 128, 512], BF16, kind="ExternalInput")
    wproj = nc.dram_tensor("wproj", [DC, 128, DC, 128], BF16, kind="ExternalInput")
    wq2 = nc.dram_tensor("wq2", [DC, 128, DC, 128], BF16, kind="ExternalInput")
    wkvk = nc.dram_tensor("wkvk", [DC, 128, DC, 128], BF16, kind="ExternalInput")
    wkvv = nc.dram_tensor("wkvv", [2, DC, 128, 512], BF16, kind="ExternalInput")
    wco = nc.dram_tensor("wco", [DC, 128, DC, 128], BF16, kind="ExternalInput")
    wm1 = nc.dram_tensor("wm1", [MC, 128, DC, 128], BF16, kind="ExternalInput")
    wm2 = nc.dram_tensor("wm2", [DC, 128, MC, 128], BF16, kind="ExternalInput")
    bq = nc.dram_tensor("bq", [DC, 128], F32, kind="ExternalInput")
    bk = nc.dram_tensor("bk", [DC, 128], F32, kind="ExternalInput")
    bv = nc.dram_tensor("bv", [1, D], BF16, kind="ExternalInput")
    bproj = nc.dram_tensor("bproj", [DC, 128], F32, kind="ExternalInput")
    bq2 = nc.dram_tensor("bq2", [DC, 128], F32, kind="ExternalInput")
    bkvk = nc.dram_tensor("bkvk", [DC, 128], F32, kind="ExternalInput")
    bkvv = nc.dram_tensor("bkvv", [1, D], BF16, kind="ExternalInput")
    bco = nc.dram_tensor("bco", [DC, 128], F32, kind="ExternalInput")
    bm1 = nc.dram_tensor("bm1", [MC, 128], F32, kind="ExternalInput")
    bm2 = nc.dram_tensor("bm2", [DC, 128], F32, kind="ExternalInput")
    out_t = nc.dram_tensor("out_t", [128, DC, R], F32, kind="ExternalOutput")

    rg = [[0, 1, 2, 3], [4, 5, 6, 7]]
    AFT = mybir.ActivationFunctionType

    with tile.TileContext(nc) as tc:
        with (
            tc.tile_pool(name="persist", bufs=1) as pp,
            tc.tile_pool(name="dram", bufs=1, space="DRAM") as dram,
        ):
            # gather buffers (<=520KB so collectives take the mesh path):
            # K^T half [512 feat, 512 tok]; V half [512 tok, 8*65] ones-interleaved
            kin_s = [dram.tile([512, 512], BF16, name=f"kinS{i}") for i in range(2)]
            kout_s = [dram.tile([2048, 512], BF16, name=f"koutS{i}") for i in range(2)]
            vin_s = [dram.tile([512, VW], BF16, name=f"vinS{i}") for i in range(2)]
            vout_s = [dram.tile([2048, VW], BF16, name=f"voutS{i}") for i in range(2)]
            kin_c = [dram.tile([512, 512], BF16, name=f"kinC{i}") for i in range(2)]
            kout_c = [dram.tile([2048, 512], BF16, name=f"koutC{i}") for i in range(2)]
            vin_c = [dram.tile([512, VW], BF16, name=f"vinC{i}") for i in range(2)]
            vout_c = [dram.tile([2048, VW], BF16, name=f"voutC{i}") for i in range(2)]

            def allgather(src_d, dst_d):
                nc.gpsimd.collective_compute(
                    "AllGather", mybir.AluOpType.bypass,
                    ins=[src_d.opt()], outs=[dst_d.opt()], replica_groups=rg)

            # persistent SBUF
            x_sb = pp.tile([128, DC, R], F32)        # residual stream x^T
            x2_sb = pp.tile([128, DC, R], F32)
            enc_sb = pp.tile([128, DC, R], BF16)
            mask_sb = pp.tile([128, KC, R], BF16)
            ln1_sb = pp.tile([128, DC, R], BF16)
            qt_sb = pp.tile([128, DC, R], BF16)      # Q^T (self)
            qt2_sb = pp.tile([128, DC, R], BF16)     # Q^T (cross)
            at_sb = pp.tile([128, DC, R], BF16)      # attn out^T (reused)
            ones_rbf = pp.tile([1, 128], BF16)       # K=1 lhsT for bf16 bcasts
            ones_f32 = pp.tile([128, 128], F32)      # K=1 lhsT rows for f32 bcasts
            ones_bf_col = pp.tile([128, 1], BF16)    # lhsT for partition sums
            bias_sb = pp.tile([128, 8 * DC + MC], F32)
            bv_sb = pp.tile([1, D], BF16)
            bkvv_sb = pp.tile([1, D], BF16)
            eps_sb = pp.tile([1, 1], F32)

            nc.vector.memset(ones_rbf[:], 1.0)
            nc.vector.memset(ones_f32[:], 1.0)
            nc.vector.memset(ones_bf_col[:], 1.0)
            nc.vector.memset(eps_sb[:], EPS)

            for c in range(DC):
                nc.sync.dma_start(x_sb[:, c, :], x_t.ap()[:, c, :])
                nc.sync.dma_start(enc_sb[:, c, :], enc_t.ap()[:, c, :])
            nc.sync.dma_start(bv_sb[:], bv.ap())
            nc.sync.dma_start(bkvv_sb[:], bkvv.ap())
            bias_list = [bq, bk, bproj, bq2, bkvk, bco, bm2]
            for i, b in enumerate(bias_list):
                nc.sync.dma_start(
                    bias_sb[:, i * DC:(i + 1) * DC],
                    b.ap().rearrange("c p -> p c"),
                )
            nc.sync.dma_start(
                bias_sb[:, 7 * DC:7 * DC + MC], bm1.ap().rearrange("c p -> p c")
            )
            B_Q, B_K, B_PROJ, B_Q2, B_KVK, B_CO, B_M2 = (
                0, DC, 2 * DC, 3 * DC, 4 * DC, 5 * DC, 6 * DC)
            B_M1 = 7 * DC

            def bias_ap(base, oc):
                return bias_sb[:, base + oc:base + oc + 1]

            # ---------- helpers ----------
            def layernorm(src_sb, dst_sb, pool, psum_pool, name,
                          cs=slice(0, R), stag="stats", btag="lnb", sbufs=2,
                          bbufs=1):
                """src_sb [128, DC, R] f32 cols cs -> ln^T bf16 into dst_sb."""
                n = cs.stop - cs.start
                ps1 = psum_pool.tile([1, n], F32, tag=stag, bufs=sbufs,
                                     name=f"p1_{name}")
                ps2 = psum_pool.tile([1, n], F32, tag=stag, bufs=sbufs,
                                     name=f"p2_{name}")
                for c in range(DC):
                    xb = pool.tile([128, R], BF16, tag="lnxb", name=f"xb_{name}{c}")
                    sq = pool.tile([128, R], BF16, tag="lnsq", name=f"sq_{name}{c}")
                    nc.vector.tensor_copy(xb[:, 0:n], src_sb[:, c, cs])
                    nc.vector.tensor_tensor(sq[:, 0:n], xb[:, 0:n], xb[:, 0:n],
                                            mybir.AluOpType.mult)
                    nc.tensor.matmul(ps1[0:1, 0:n], lhsT=ones_bf_col[:],
                                     rhs=xb[:, 0:n],
                                     start=(c == 0), stop=(c == DC - 1))
                    nc.tensor.matmul(ps2[0:1, 0:n], lhsT=ones_bf_col[:],
                                     rhs=sq[:, 0:n],
                                     start=(c == 0), stop=(c == DC - 1))
                nmean = pool.tile([1, R], F32, tag="lnrow", bufs=8, name=f"nm_{name}")
                ex2 = pool.tile([1, R], F32, tag="lnrow", bufs=8, name=f"e2_{name}")
                m2 = pool.tile([1, R], F32, tag="lnrow", bufs=8, name=f"m2_{name}")
                var = pool.tile([1, R], F32, tag="lnrow", bufs=8, name=f"va_{name}")
                lnv = pool.tile([1, R], F32, tag="lnrow", bufs=8, name=f"lv_{name}")
                rstd = pool.tile([1, R], F32, tag="lnrow", bufs=8, name=f"rs_{name}")
                nmrs = pool.tile([1, R], F32, tag="lnrow", bufs=8, name=f"nr_{name}")
                nc.scalar.activation(nmean[0:1, 0:n], ps1[0:1, 0:n], AFT.Identity,
                                     scale=-1.0 / D)
                nc.scalar.activation(ex2[0:1, 0:n], ps2[0:1, 0:n], AFT.Identity,
                                     scale=1.0 / D)
                nc.vector.tensor_tensor(m2[0:1, 0:n], nmean[0:1, 0:n],
                                        nmean[0:1, 0:n], mybir.AluOpType.mult)
                nc.vector.tensor_tensor(var[0:1, 0:n], ex2[0:1, 0:n],
                                        m2[0:1, 0:n], mybir.AluOpType.subtract)
                # rstd = exp(-0.5 * ln(var + eps)) — stays in the exp/ln set
                nc.scalar.activation(lnv[0:1, 0:n], var[0:1, 0:n], AFT.Ln,
                                     bias=eps_sb[:])
                nc.scalar.activation(rstd[0:1, 0:n], lnv[0:1, 0:n], AFT.Exp,
                                     scale=-0.5)
                nc.vector.tensor_tensor(nmrs[0:1, 0:n], nmean[0:1, 0:n],
                                        rstd[0:1, 0:n], mybir.AluOpType.mult)
                psb = psum_pool.tile([128, 2, n], F32, tag=btag, bufs=bbufs,
                                     name=f"pb_{name}")
                nc.tensor.matmul(psb[:, 0, 0:n], lhsT=ones_f32[0:1, :],
                                 rhs=rstd[0:1, 0:n], start=True, stop=True)
                nc.tensor.matmul(psb[:, 1, 0:n], lhsT=ones_f32[0:1, :],
                                 rhs=nmrs[0:1, 0:n], start=True, stop=True)
                tmp = pool.tile([128, R], F32, tag="lntmp", name=f"tp_{name}")
                for c in range(DC):
                    nc.vector.tensor_tensor(tmp[:, 0:n], src_sb[:, c, cs],
                                            psb[:, 0, 0:n],
                                            mybir.AluOpType.mult)
                    nc.vector.tensor_tensor(dst_sb[:, c, cs], tmp[:, 0:n],
                                            psb[:, 1, 0:n],
                                            mybir.AluOpType.add)

            def matmul_t(rhs_sb, w_dram, n_k, n_o, pool, psum_pool, name,
                         consume, w_tag="wtile", ocs=None, ps_bufs=3):
                """out^T[oc] = sum_kc W[oc][kc].T @ rhs[kc]; consume(oc, psum)."""
                for oc in (range(n_o) if ocs is None else ocs):
                    wt = pool.tile([128, n_k, 128], BF16, tag=w_tag,
                                   name=f"w_{name}_{oc}")
                    nc.sync.dma_start(wt[:], w_dram.ap()[oc])
                    ps = psum_pool.tile([128, R], F32, tag="mm", bufs=ps_bufs,
                                        name=f"ps_{name}_{oc}")
                    for kc in range(n_k):
                        nc.tensor.matmul(ps[:], lhsT=wt[:, kc, :],
                                         rhs=rhs_sb[:, kc, :],
                                         start=(kc == 0), stop=(kc == n_k - 1))
                    consume(oc, ps)

            def kv_half(src_sb, wk_d, wv_d, bk_base, bv_row, kin, kout, vin,
                        vout, pool, psum_pool, half, name):
                """K^T half + V half (with ones cols) -> gather buffers + AGs."""
                # K^T: 4 output chunks = features [512*half, 512*half+512)
                def eat_k(oc, ps):
                    ocl = oc - half * (DC // 2)
                    kl = pool.tile([128, R], BF16, tag="kvcopy", name=f"kl_{name}{oc}")
                    nc.vector.tensor_scalar_add(kl[:], ps[:], bias_ap(bk_base, oc))
                    nc.sync.dma_start(
                        kin[ocl * 128:(ocl + 1) * 128, :], kl[:])

                matmul_t(src_sb, wk_d, DC, DC, pool, psum_pool, f"k{name}",
                         eat_k, ocs=range(half * (DC // 2), (half + 1) * (DC // 2)),
                         ps_bufs=4)
                allgather(kin, kout)

                # V: rows x 512 features of this half, + interleaved ones.
                # Weight tile loaded once per kc; 4 row-chunk psums accumulate.
                psvs = [psum_pool.tile([128, 512], F32, tag="mm", bufs=4,
                                       name=f"psv_{name}_{rc}")
                        for rc in range(R // 128)]
                for kc in range(DC):
                    wt = pool.tile([128, 512], BF16, tag="wv",
                                   name=f"wv_{name}_{kc}")
                    nc.sync.dma_start(wt[:], wv_d.ap()[half, kc])
                    for rc in range(R // 128):
                        nc.tensor.matmul(
                            psvs[rc][:],
                            lhsT=src_sb[:, kc, rc * 128:(rc + 1) * 128],
                            rhs=wt[:], start=(kc == 0), stop=False)
                for rc in range(R // 128):
                    nc.tensor.matmul(psvs[rc][:], lhsT=ones_rbf[:],
                                     rhs=bv_row[:, half * 512:(half + 1) * 512],
                                     start=False, stop=True)
                    vl = pool.tile([128, 8, HD + 1], BF16, tag="vcopy",
                                   name=f"vl_{name}_{rc}")
                    nc.vector.tensor_copy(
                        vl[:, :, 0:HD],
                        psvs[rc][:].rearrange("p (h d) -> p h d", d=HD))
                    nc.vector.memset(vl[:, :, HD:HD + 1], 1.0)
                    nc.sync.dma_start(
                        vin[rc * 128:(rc + 1) * 128, :],
                        vl[:].rearrange("p h d -> p (h d)"))
                allgather(vin, vout)

            def attention(qt, kouts, vouts, dst_sb, masked, pool, psum_pool,
                          name):
                """dst_sb [128, DC, R] bf16 = attn(Q^T, gathered K/V)^T."""
                for hp in range(NPAIR):
                    half, hpl = hp // (NPAIR // 2), hp % (NPAIR // 2)
                    ko, vo = kouts[half], vouts[half]
                    # K^T tile: [feat 128, kc, key 128]
                    ktp = pool.tile([128, KC, 128], BF16, tag="ktp",
                                    name=f"kt_{name}_{hp}")
                    vt = pool.tile([128, KC, 2 * (HD + 1)], BF16, tag="vt",
                                   name=f"v_{name}_{hp}")
                    j0 = hpl * 2 * (HD + 1)
                    for s in range(4):
                        f0 = 512 * s + 128 * hpl
                        nc.sync.dma_start(
                            ktp[:, 4 * s:4 * s + 4, :],
                            ko[f0:f0 + 128, :]
                            .rearrange("p (c m) -> p c m", m=128))
                        # V tile: [key 128, kc, 2*(HD+1)] heads + ones cols
                        nc.sync.dma_start(
                            vt[:, 4 * s:4 * s + 4, :],
                            vo[512 * s:512 * s + 512, j0:j0 + 2 * (HD + 1)]
                            .rearrange("(c p) w -> p c w", p=128))
                    psA = psum_pool.tile([HD + 1, R], F32, tag="psO", bufs=2,
                                         name=f"oA_{name}_{hp}")
                    psB = psum_pool.tile([HD + 1, R], F32, tag="psO", bufs=2,
                                         name=f"oB_{name}_{hp}")
                    for kc in range(KC):
                        pss = psum_pool.tile([128, 2, R], F32, tag="psS", bufs=3,
                                             name=f"s_{name}_{hp}_{kc}")
                        nc.tensor.matmul(pss[:, 0, :],
                                         lhsT=ktp[0:64, kc, :],
                                         rhs=qt[0:64, hp, :],
                                         start=True, stop=True)
                        nc.tensor.matmul(pss[:, 1, :],
                                         lhsT=ktp[64:128, kc, :],
                                         rhs=qt[64:128, hp, :],
                                         start=True, stop=True)
                        es = pool.tile([128, 2, R], BF16, tag="expS", bufs=3,
                                       name=f"e_{name}_{hp}_{kc}")
                        nc.scalar.activation(
                            es[:].rearrange("p a f -> p (a f)"),
                            pss[:].rearrange("p a f -> p (a f)"),
                            AFT.Exp, scale=SCALE)
                        if masked:
                            nc.vector.tensor_tensor(
                                es[:], es[:],
                                mask_sb[:, kc, None, :].to_broadcast((128, 2, R)),
                                mybir.AluOpType.mult)
                        nc.tensor.matmul(psA[:], lhsT=vt[:, kc, 0:HD + 1],
                                         rhs=es[:, 0, :],
                                         start=(kc == 0), stop=(kc == KC - 1))
                        nc.tensor.matmul(psB[:], lhsT=vt[:, kc, HD + 1:],
                                         rhs=es[:, 1, :],
                                         start=(kc == 0), stop=(kc == KC - 1))
                    # normalize: copy raw denom, PE-broadcast to 64 lanes,
                    # reciprocal, multiply
                    rec = pool.tile([HD + 1, R], F32, tag="rec", bufs=4,
                                    name=f"r_{name}_{hp}")
                    nc.vector.tensor_copy(rec[HD:HD + 1, 0:R],
                                          psA[HD:HD + 1, :])
                    recB = pool.tile([HD + 1, R], F32, tag="recB", bufs=4,
                                     name=f"rB_{name}_{hp}")
                    nc.vector.tensor_copy(recB[HD:HD + 1, 0:R],
                                          psB[HD:HD + 1, :])
                    psn = psum_pool.tile([128, 2, R], F32, tag="psS", bufs=3,
                                         name=f"n_{name}_{hp}")
                    nc.tensor.matmul(psn[0:HD, 0, :],
                                     lhsT=ones_f32[HD:HD + 1, 0:HD],
                                     rhs=rec[HD:HD + 1, :],
                                     start=True, stop=True)
                    nc.tensor.matmul(psn[0:HD, 1, :],
                                     lhsT=ones_f32[HD:HD + 1, 0:HD],
                                     rhs=recB[HD:HD + 1, :],
                                     start=True, stop=True)
                    bcs = pool.tile([HD, 2, R], F32, tag="bcs",
                                    name=f"c_{name}_{hp}")
                    nc.vector.reciprocal_approx_fast(bcs[:, 0, :],
                                                     psn[0:HD, 0, :])
                    nc.vector.reciprocal_approx_fast(bcs[:, 1, :],
                                                     psn[0:HD, 1, :])
                    nc.vector.tensor_tensor(
                        dst_sb[0:HD, hp, :], psA[0:HD, :], bcs[:, 0, :],
                        mybir.AluOpType.mult)
                    tmb = pool.tile([HD, R], BF16, tag="tmb",
                                    name=f"t_{name}_{hp}")
                    nc.vector.tensor_tensor(tmb[:], psB[0:HD, :],
                                            bcs[:, 1, :],
                                            mybir.AluOpType.mult)
                    nc.sync.dma_start(dst_sb[HD:128, hp, :], tmb[:])

            # ============ phase 1: ln1, K/V(+gathers), Q, cross K/V ==========
            with (
                tc.tile_pool(name="p1", bufs=2) as pool,
                tc.tile_pool(name="p1ps", bufs=2, space="PSUM") as psum_pool,
            ):
                layernorm(x_sb, ln1_sb, pool, psum_pool, "ln1")
                kv_half(ln1_sb, wk, wv, B_K, bv_sb, kin_s[0], kout_s[0],
                        vin_s[0], vout_s[0], pool, psum_pool, 0, "sA")
                kv_half(ln1_sb, wk, wv, B_K, bv_sb, kin_s[1], kout_s[1],
                        vin_s[1], vout_s[1], pool, psum_pool, 1, "sB")

                def eat_q(oc, ps):
                    nc.scalar.activation(qt_sb[:, oc, :], ps[:], AFT.Identity,
                                         bias=bias_ap(B_Q, oc))

                matmul_t(ln1_sb, wq, DC, DC, pool, psum_pool, "q", eat_q,
                         ps_bufs=4)

                kv_half(enc_sb, wkvk, wkvv, B_KVK, bkvv_sb, kin_c[0],
                        kout_c[0], vin_c[0], vout_c[0], pool, psum_pool,
                        0, "cA")
                kv_half(enc_sb, wkvk, wkvv, B_KVK, bkvv_sb, kin_c[1],
                        kout_c[1], vin_c[1], vout_c[1], pool, psum_pool,
                        1, "cB")

            nc.sync.dma_start(mask_sb[:], mask_t.ap())
            if dbg:
                nc.sync.dma_start(dbg_t["ln1"].ap(), ln1_sb[:])
                nc.sync.dma_start(dbg_t["qt"].ap(), qt_sb[:])

            # ============ phase 2: self attention ============================
            with (
                tc.tile_pool(name="p2", bufs=2) as pool,
                tc.tile_pool(name="p2ps", bufs=2, space="PSUM") as psum_pool,
            ):
                attention(qt_sb, kout_s, vout_s, at_sb, True, pool, psum_pool,
                          "sa")
            if dbg:
                nc.sync.dma_start(dbg_t["at"].ap(), at_sb[:])

            # ============ phase 3: proj + residual, ln2, q2 ==================
            with (
                tc.tile_pool(name="p3", bufs=2) as pool,
                tc.tile_pool(name="p3ps", bufs=2, space="PSUM") as psum_pool,
            ):
                def eat_proj(oc, ps):
                    nc.vector.scalar_tensor_tensor(
                        x2_sb[:, oc, :], ps[:], bias_ap(B_PROJ, oc),
                        x_sb[:, oc, :],
                        mybir.AluOpType.add, mybir.AluOpType.add)

                matmul_t(at_sb, wproj, DC, DC, pool, psum_pool, "pr", eat_proj)
                if dbg:
                    nc.sync.dma_start(dbg_t["x2"].ap(), x2_sb[:])
                layernorm(x2_sb, ln1_sb, pool, psum_pool, "ln2")

                def eat_q2(oc, ps):
                    nc.scalar.activation(qt2_sb[:, oc, :], ps[:], AFT.Identity,
                                         bias=bias_ap(B_Q2, oc))

                matmul_t(ln1_sb, wq2, DC, DC, pool, psum_pool, "q2", eat_q2)

            # ============ phase 4: cross attention ===========================
            with (
                tc.tile_pool(name="p4", bufs=2) as pool,
                tc.tile_pool(name="p4ps", bufs=2, space="PSUM") as psum_pool,
            ):
                attention(qt2_sb, kout_c, vout_c, at_sb, False, pool,
                          psum_pool, "ca")

            # ============ phase 5: co + residual, ln3, MLP ===================
            with (
                tc.tile_pool(name="p5", bufs=2) as pool,
                tc.tile_pool(name="p5ps", bufs=2, space="PSUM") as psum_pool,
            ):
                def eat_co(oc, ps):
                    nc.vector.scalar_tensor_tensor(
                        x_sb[:, oc, :], ps[:], bias_ap(B_CO, oc),
                        x2_sb[:, oc, :],
                        mybir.AluOpType.add, mybir.AluOpType.add)

                matmul_t(at_sb, wco, DC, DC, pool, psum_pool, "co", eat_co)
                layernorm(x_sb, ln1_sb, pool, psum_pool, "ln3")

                h_sb = pool.tile([128, MC, R], BF16, tag="hsb", bufs=1)

                def eat_m1(oc, ps):
                    nc.scalar.activation(h_sb[:, oc, :], ps[:], AFT.Gelu,
                                         bias=bias_ap(B_M1, oc))

                matmul_t(ln1_sb, wm1, DC, MC, pool, psum_pool, "m1", eat_m1)

                def eat_m2(oc, ps):
                    nc.vector.scalar_tensor_tensor(
                        x2_sb[:, oc, :], ps[:], bias_ap(B_M2, oc),
                        x_sb[:, oc, :],
                        mybir.AluOpType.add, mybir.AluOpType.add)
                    nc.sync.dma_start(out_t.ap()[:, oc, :], x2_sb[:, oc, :])

                matmul_t(h_sb, wm2, MC, DC, pool, psum_pool, "m2", eat_m2,
                         w_tag="wtile2")

    nc.finalize()
    return nc


def zig_rows(g):
    """Zigzag row assignment: core g owns [256g,256g+256) u [1024+256g,+256)."""
    return np.concatenate([np.arange(256 * g, 256 * g + 256),
                           np.arange(1024 + 256 * g, 1024 + 256 * g + 256)])


def prep_inputs(inputs):
    """Host-side prep: fold LN affine into weights, cast/tile, shard rows."""
    f32 = np.float32
    bf16 = ml_dtypes.bfloat16

    def tile_w(w, nk, no):
        # [nk*128, no*128] -> [no, 128, nk, 128] (contiguous per-oc tiles)
        return np.ascontiguousarray(
            w.reshape(nk, 128, no, 128).transpose(2, 1, 0, 3)).astype(bf16)

    def tile_v(w):
        # [D, D] -> [2, DC, 128, 512] (contiguous [128, 512] tiles per half)
        return np.ascontiguousarray(
            w.reshape(DC, 128, 2, 512).transpose(2, 0, 1, 3)).astype(bf16)

    def chunk_b(b, n):
        return np.ascontiguousarray(b.reshape(n, 128)).astype(f32)

    def chunk_t(a):
        # [rows, D] -> [128, DC, rows] transposed chunked
        return np.ascontiguousarray(
            a.T.reshape(DC, 128, -1).transpose(1, 0, 2))

    x = np.asarray(inputs["x"], f32)
    enc = np.asarray(inputs["enc_out"], f32)
    cm = np.asarray(inputs["causal_mask"])

    ln1_g, ln1_b = np.asarray(inputs["ln1_g"], f32), np.asarray(inputs["ln1_b"], f32)
    ln2_g, ln2_b = np.asarray(inputs["ln2_g"], f32), np.asarray(inputs["ln2_b"], f32)
    ln3_g, ln3_b = np.asarray(inputs["ln3_g"], f32), np.asarray(inputs["ln3_b"], f32)
    qkv_w = np.asarray(inputs["qkv_w"], f32)
    qkv_b = np.asarray(inputs["qkv_b"], f32)
    q_w, q_b = np.asarray(inputs["q_w"], f32), np.asarray(inputs["q_b"], f32)
    kv_w, kv_b = np.asarray(inputs["kv_w"], f32), np.asarray(inputs["kv_b"], f32)
    mlp1_w, mlp1_b = np.asarray(inputs["mlp1_w"], f32), np.asarray(inputs["mlp1_b"], f32)

    qkv_w_eff = ln1_g[:, None] * qkv_w
    qkv_b_eff = qkv_b + ln1_b @ qkv_w
    q_w_eff = ln2_g[:, None] * q_w
    q_b_eff = q_b + ln2_b @ q_w
    m1_w_eff = ln3_g[:, None] * mlp1_w
    m1_b_eff = mlp1_b + ln3_b @ mlp1_w

    shared = {
        "wq": tile_w(qkv_w_eff[:, 0:D], DC, DC),
        "wk": tile_w(qkv_w_eff[:, D:2 * D], DC, DC),
        "wv": tile_v(qkv_w_eff[:, 2 * D:3 * D]),
        "wproj": tile_w(np.asarray(inputs["proj_w"], f32), DC, DC),
        "wq2": tile_w(q_w_eff, DC, DC),
        "wkvk": tile_w(kv_w[:, 0:D], DC, DC),
        "wkvv": tile_v(kv_w[:, D:2 * D]),
        "wco": tile_w(np.asarray(inputs["co_w"], f32), DC, DC),
        "wm1": tile_w(m1_w_eff, DC, MC),
        "wm2": tile_w(np.asarray(inputs["mlp2_w"], f32), MC, DC),
        "bq": chunk_b(qkv_b_eff[0:D], DC),
        "bk": chunk_b(qkv_b_eff[D:2 * D], DC),
        "bv": qkv_b_eff[2 * D:3 * D].reshape(1, D).astype(bf16),
        "bproj": chunk_b(np.asarray(inputs["proj_b"], f32), DC),
        "bq2": chunk_b(q_b_eff, DC),
        "bkvk": chunk_b(kv_b[0:D], DC),
        "bkvv": kv_b[D:2 * D].reshape(1, D).astype(bf16),
        "bco": chunk_b(np.asarray(inputs["co_b"], f32), DC),
        "bm1": chunk_b(m1_b_eff, MC),
        "bm2": chunk_b(np.asarray(inputs["mlp2_b"], f32), DC),
    }

    mask_f = (cm != 0).astype(f32)
    in_maps = []
    for c in range(N_CORES):
        b = c // GROUP
        r0 = (c % GROUP) * R
        m = dict(shared)
        m["x_t"] = chunk_t(x[b, r0:r0 + R]).astype(f32)
        m["enc_t"] = chunk_t(enc[b, r0:r0 + R]).astype(bf16)
        m["mask_t"] = np.ascontiguousarray(
            mask_f[r0:r0 + R].T.reshape(KC, 128, R).transpose(1, 0, 2)
        ).astype(bf16)
        in_maps.append(m)
    return in_maps


_prog_cache = {}


def kernel(**inputs):
    if "nc" not in _prog_cache:
        _prog_cache["nc"] = build_program()
    nc = _prog_cache["nc"]
    in_maps = prep_inputs(inputs)
    res = run_bass_kernel_spmd(nc, in_maps, core_ids=list(range(N_CORES)))
    out = np.empty((B, T, D), np.float32)
    for c in range(N_CORES):
        b = c // GROUP
        r0 = (c % GROUP) * R
        # out_t [128, DC, R] -> [R, D]
        ot = res.results[c]["out_t"]
        out[b, r0:r0 + R] = ot.transpose(2, 1, 0).reshape(R, D)
    _prog_cache["last_results"] = res
    return out


# revision 51
# speedup vs baseline: 1.1455x; 1.0293x over previous
"""Trainium2 Bass kernel for nn_DecoderBlock (B=2, T=2048, D=1024, H=16, MLP=4096).

Sharding: sequence/row parallel over 8 cores (4 cores per batch, 512 rows each).
K/V for both attentions are computed on local rows and AllGathered (2 groups of
4 cores).  K and V are packed into ONE gather buffer per head-half (4 gathers
total), launched as early as possible so they overlap Q / cross-KV compute.

On-device layout: activations transposed [feature, row]; LayerNorm affine
params folded into the following weight matrix on the host.  Weights are
pre-tiled on the host so every SBUF weight tile is one contiguous DRAM block
(fat DMA descriptors).  Softmax without max-subtraction; mask applied as 0/1
multiply on exp(scores); denominator comes from a ones column interleaved with
V in the gather buffer ([r, 8*65] layout).  LN rstd via exp(-0.5*ln(var+eps))
so the whole kernel uses one ACT table set until the final Gelu.
"""

import sys

if "/opt/trn_rl_repo" not in sys.path:
    sys.path.insert(0, "/opt/trn_rl_repo")

import numpy as np
import ml_dtypes

import concourse.bass as bass
import concourse.mybir as mybir
import concourse.tile as tile
from concourse import bacc
from concourse.bass_utils import run_bass_kernel_spmd

F32 = mybir.dt.float32
BF16 = mybir.dt.bfloat16

B, T, D, H, HD = 2, 2048, 1024, 16, 64
MLP = 4 * D
EPS = 1e-5
N_CORES = 8
GROUP = 4            # cores per batch element
R = T // GROUP       # rows per core = 512
DC = D // 128        # feature chunks = 8
MC = MLP // 128      # mlp chunks = 32
KC = T // 128        # key chunks = 16
NPAIR = H // 2       # head pairs = 8
SCALE = HD ** -0.5
VW = 8 * (HD + 1)    # V gather row width (ones interleaved) = 520
GW = 512 + VW        # gather buffer row width = 1032
NB = 256             # row block (2 zigzag blocks per core)


def build_program(dbg=False):
    nc = bacc.Bacc("TRN2", target_bir_lowering=False, debug=False,
                   num_devices=N_CORES)
    dbg_t = {}
    if dbg:
        dbg_t["ln1"] = nc.dram_tensor("d_ln1", [128, DC, R], BF16,
                                      kind="ExternalOutput")
        dbg_t["qt"] = nc.dram_tensor("d_qt", [128, DC, R], BF16,
                                     kind="ExternalOutput")
        dbg_t["at"] = nc.dram_tensor("d_at", [128, DC, R], BF16,
                                     kind="ExternalOutput")
        dbg_t["x2"] = nc.dram_tensor("d_x2", [128, DC, R], F32,
                                     kind="ExternalOutput")
        dbg_t["pss0"] = nc.dram_tensor("d_pss0", [128, 2, R], F32,
                                       kind="ExternalOutput")
        dbg_t["es0"] = nc.dram_tensor("d_es0", [128, 2, R], BF16,
                                      kind="ExternalOutput")
        dbg_t["psA"] = nc.dram_tensor("d_psA", [HD + 1, R], F32,
                                      kind="ExternalOutput")
        dbg_t["psB"] = nc.dram_tensor("d_psB", [HD + 1, R], F32,
                                      kind="ExternalOutput")
        dbg_t["bcs"] = nc.dram_tensor("d_bcs", [HD, 2, R], F32,
                                      kind="ExternalOutput")

    # ---- DRAM I/O ----
    x_t = nc.dram_tensor("x_t", [128, DC, R], F32, kind="ExternalInput")
    enc_t = nc.dram_tensor("enc_t", [128, DC, R], BF16, kind="ExternalInput")
    mask_t = nc.dram_tensor("mask_t", [128, KC, R], BF16, kind="ExternalInput")
    # dense weights pre-tiled [n_out, 128, n_k, 128]
    wq = nc.dram_tensor("wq", [DC, 128, DC, 128], BF16, kind="ExternalInput")
    wk = nc.dram_tensor("wk", [DC, 128, DC, 128], BF16, kind="ExternalInput")
    wv = nc.dram_tensor("wv", [2, DC, 128, 512], BF16, kind="ExternalInput")
    wproj = nc.dram_tensor("wproj", [DC, 128, DC, 128], BF16, kind="ExternalInput")
    wq2 = nc.dram_tensor("wq2", [DC, 128, DC, 128], BF16, kind="ExternalInput")
    wkvk = nc.dram_tensor("wkvk", [DC, 128, DC, 128], BF16, kind="ExternalInput")
    wkvv = nc.dram_tensor("wkvv", [2, DC, 128, 512], BF16, kind="ExternalInput")
    wco = nc.dram_tensor("wco", [DC, 128, DC, 128], BF16, kind="ExternalInput")
    wm1 = nc.dram_tensor("wm1", [MC, 128, DC, 128], BF16, kind="ExternalInput")
    wm2 = nc.dram_tensor("wm2", [DC, 128, MC, 128], BF16, kind="ExternalInput")
    bq = nc.dram_tensor("bq", [DC, 128], F32, kind="ExternalInput")
    bk = nc.dram_tensor("bk", [DC, 128], F32, kind="ExternalInput")
    bv = nc.dram_tensor("bv", [1, D], BF16, kind="ExternalInput")
    bproj = nc.dram_tensor("bproj", [DC, 128], F32, kind="ExternalInput")
    bq2 = nc.dram_tensor("bq2", [DC, 128], F32, kind="ExternalInput")
    bkvk = nc.dram_tensor("bkvk", [DC, 128], F32, kind="ExternalInput")
    bkvv = nc.dram_tensor("bkvv", [1, D], BF16, kind="ExternalInput")
    bco = nc.dram_tensor("bco", [DC, 128], F32, kind="ExternalInput")
    bm1 = nc.dram_tensor("bm1", [MC, 128], F32, kind="ExternalInput")
    bm2 = nc.dram_tensor("bm2", [DC, 128], F32, kind="ExternalInput")
    out_t = nc.dram_tensor("out_t", [128, DC, R], F32, kind="ExternalOutput")

    rg = [[0, 1, 2, 3], [4, 5, 6, 7]]
    AFT = mybir.ActivationFunctionType

    with tile.TileContext(nc) as tc:
        with (
            tc.tile_pool(name="persist", bufs=1) as pp,
            tc.tile_pool(name="dram", bufs=1, space="DRAM") as dram,
        ):
            # gather buffers (<=520KB so collectives take the mesh path):
            # K^T half [512 feat, 512 tok]; V half [512 tok, 8*65] ones-interleaved
            kin_s = [dram.tile([512, 512], BF16, name=f"kinS{i}") for i in range(2)]
            kout_s = [dram.tile([2048, 512], BF16, name=f"koutS{i}") for i in range(2)]
            vin_s = [dram.tile([512, VW], BF16, name=f"vinS{i}") for i in range(2)]
            vout_s = [dram.tile([2048, VW], BF16, name=f"voutS{i}") for i in range(2)]
            kin_c = [dram.tile([512, 512], BF16, name=f"kinC{i}") for i in range(2)]
            kout_c = [dram.tile([2048, 512], BF16, name=f"koutC{i}") for i in range(2)]
            vin_c = [dram.tile([512, VW], BF16, name=f"vinC{i}") for i in range(2)]
            vout_c = [dram.tile([2048, VW], BF16, name=f"voutC{i}") for i in range(2)]

            def allgather(src_d, dst_d):
                nc.gpsimd.collective_compute(
                    "AllGather", mybir.AluOpType.bypass,
                    ins=[src_d.opt()], outs=[dst_d.opt()], replica_groups=rg)

            # persistent SBUF
            x_sb = pp.tile([128, DC, R], F32)        # residual stream x^T
            x2_sb = pp.tile([128, DC, R], F32)
            enc_sb = pp.tile([128, DC, R], BF16)
            mask_sb = pp.tile([128, KC, R], BF16)
            ln1_sb = pp.tile([128, DC, R], BF16)
            qt_sb = pp.tile([128, DC, R], BF16)      # Q^T (self)
            qt2_sb = pp.tile([128, DC, R], BF16)     # Q^T (cross)
            at_sb = pp.tile([128, DC, R], BF16)      # attn out^T (reused)
            ones_rbf = pp.tile([1, 128], BF16)       # K=1 lhsT for bf16 bcasts
            ones_f32 = pp.tile([128, 128], F32)      # K=1 lhsT rows for f32 bcasts
            ones_bf_col = pp.tile([128, 1], BF16)    # lhsT for partition sums
            bias_sb = pp.tile([128, 8 * DC + MC], F32)
            bv_sb = pp.tile([1, D], BF16)
            bkvv_sb = pp.tile([1, D], BF16)
            eps_sb = pp.tile([1, 1], F32)

            nc.vector.memset(ones_rbf[:], 1.0)
            nc.vector.memset(ones_f32[:], 1.0)
            nc.vector.memset(ones_bf_col[:], 1.0)
            nc.vector.memset(eps_sb[:], EPS)

            for c in range(DC):
                nc.sync.dma_start(x_sb[:, c, :], x_t.ap()[:, c, :])
                nc.sync.dma_start(enc_sb[:, c, :], enc_t.ap()[:, c, :])
            nc.sync.dma_start(bv_sb[:], bv.ap())
            nc.sync.dma_start(bkvv_sb[:], bkvv.ap())
            bias_list = [bq, bk, bproj, bq2, bkvk, bco, bm2]
            for i, b in enumerate(bias_list):
                nc.sync.dma_start(
                    bias_sb[:, i * DC:(i + 1) * DC],
                    b.ap().rearrange("c p -> p c"),
                )
            nc.sync.dma_start(
                bias_sb[:, 7 * DC:7 * DC + MC], bm1.ap().rearrange("c p -> p c")
            )
            B_Q, B_K, B_PROJ, B_Q2, B_KVK, B_CO, B_M2 = (
                0, DC, 2 * DC, 3 * DC, 4 * DC, 5 * DC, 6 * DC)
            B_M1 = 7 * DC

            def bias_ap(base, oc):
                return bias_sb[:, base + oc:base + oc + 1]

            # ---------- helpers ----------
            def layernorm(src_sb, dst_sb, pool, psum_pool, name,
                          cs=slice(0, R), stag="stats", btag="lnb", sbufs=2,
                          bbufs=1):
                """src_sb [128, DC, R] f32 cols cs -> ln^T bf16 into dst_sb."""
                n = cs.stop - cs.start
                ps1 = psum_pool.tile([1, n], F32, tag=stag, bufs=sbufs,
                                     name=f"p1_{name}")
                ps2 = psum_pool.tile([1, n], F32, tag=stag, bufs=sbufs,
                                     name=f"p2_{name}")
                for c in range(DC):
                    xb = pool.tile([128, R], BF16, tag="lnxb", name=f"xb_{name}{c}")
                    sq = pool.tile([128, R], BF16, tag="lnsq", name=f"sq_{name}{c}")
                    nc.vector.tensor_copy(xb[:, 0:n], src_sb[:, c, cs])
                    nc.vector.tensor_tensor(sq[:, 0:n], xb[:, 0:n], xb[:, 0:n],
                                            mybir.AluOpType.mult)
                    nc.tensor.matmul(ps1[0:1, 0:n], lhsT=ones_bf_col[:],
                                     rhs=xb[:, 0:n],
                                     start=(c == 0), stop=(c == DC - 1))
                    nc.tensor.matmul(ps2[0:1, 0:n], lhsT=ones_bf_col[:],
                                     rhs=sq[:, 0:n],
                                     start=(c == 0), stop=(c == DC - 1))
                nmean = pool.tile([1, R], F32, tag="lnrow", bufs=8, name=f"nm_{name}")
                ex2 = pool.tile([1, R], F32, tag="lnrow", bufs=8, name=f"e2_{name}")
                m2 = pool.tile([1, R], F32, tag="lnrow", bufs=8, name=f"m2_{name}")
                var = pool.tile([1, R], F32, tag="lnrow", bufs=8, name=f"va_{name}")
                lnv = pool.tile([1, R], F32, tag="lnrow", bufs=8, name=f"lv_{name}")
                rstd = pool.tile([1, R], F32, tag="lnrow", bufs=8, name=f"rs_{name}")
                nmrs = pool.tile([1, R], F32, tag="lnrow", bufs=8, name=f"nr_{name}")
                nc.scalar.activation(nmean[0:1, 0:n], ps1[0:1, 0:n], AFT.Identity,
                                     scale=-1.0 / D)
                nc.scalar.activation(ex2[0:1, 0:n], ps2[0:1, 0:n], AFT.Identity,
                                     scale=1.0 / D)
                nc.vector.tensor_tensor(m2[0:1, 0:n], nmean[0:1, 0:n],
                                        nmean[0:1, 0:n], mybir.AluOpType.mult)
                nc.vector.tensor_tensor(var[0:1, 0:n], ex2[0:1, 0:n],
                                        m2[0:1, 0:n], mybir.AluOpType.subtract)
                # rstd = exp(-0.5 * ln(var + eps)) — stays in the exp/ln set
                nc.scalar.activation(lnv[0:1, 0:n], var[0:1, 0:n], AFT.Ln,
                                     bias=eps_sb[:])
                nc.scalar.activation(rstd[0:1, 0:n], lnv[0:1, 0:n], AFT.Exp,
                                     scale=-0.5)
                nc.vector.tensor_tensor(nmrs[0:1, 0:n], nmean[0:1, 0:n],
                                        rstd[0:1, 0:n], mybir.AluOpType.mult)
                psb = psum_pool.tile([128, 2, n], F32, tag=btag, bufs=bbufs,
                                     name=f"pb_{name}")
                nc.tensor.matmul(psb[:, 0, 0:n], lhsT=ones_f32[0:1, :],
                                 rhs=rstd[0:1, 0:n], start=True, stop=True)
                nc.tensor.matmul(psb[:, 1, 0:n], lhsT=ones_f32[0:1, :],
                                 rhs=nmrs[0:1, 0:n], start=True, stop=True)
                tmp = pool.tile([128, R], F32, tag="lntmp", name=f"tp_{name}")
                for c in range(DC):
                    nc.vector.tensor_tensor(tmp[:, 0:n], src_sb[:, c, cs],
                                            psb[:, 0, 0:n],
                                            mybir.AluOpType.mult)
                    nc.vector.tensor_tensor(dst_sb[:, c, cs], tmp[:, 0:n],
                                            psb[:, 1, 0:n],
                                            mybir.AluOpType.add)

            def matmul_t(rhs_sb, w_dram, n_k, n_o, pool, psum_pool, name,
                         consume, w_tag="wtile", ocs=None, ps_bufs=3):
                """out^T[oc] = sum_kc W[oc][kc].T @ rhs[kc]; consume(oc, psum)."""
                for oc in (range(n_o) if ocs is None else ocs):
                    wt = pool.tile([128, n_k, 128], BF16, tag=w_tag,
                                   name=f"w_{name}_{oc}")
                    nc.sync.dma_start(wt[:], w_dram.ap()[oc])
                    ps = psum_pool.tile([128, R], F32, tag="mm", bufs=ps_bufs,
                                        name=f"ps_{name}_{oc}")
                    for kc in range(n_k):
                        nc.tensor.matmul(ps[:], lhsT=wt[:, kc, :],
                                         rhs=rhs_sb[:, kc, :],
                                         start=(kc == 0), stop=(kc == n_k - 1))
                    consume(oc, ps)

            def kv_half(src_sb, wk_d, wv_d, bk_base, bv_row, kin, kout, vin,
                        vout, pool, psum_pool, half, name):
                """K^T half + V half (with ones cols) -> gather buffers + AGs."""
                # K^T: 4 output chunks = features [512*half, 512*half+512)
                def eat_k(oc, ps):
                    ocl = oc - half * (DC // 2)
                    kl = pool.tile([128, R], BF16, tag="kvcopy", name=f"kl_{name}{oc}")
                    nc.vector.tensor_scalar_add(kl[:], ps[:], bias_ap(bk_base, oc))
                    nc.sync.dma_start(
                        kin[ocl * 128:(ocl + 1) * 128, :], kl[:])

                matmul_t(src_sb, wk_d, DC, DC, pool, psum_pool, f"k{name}",
                         eat_k, ocs=range(half * (DC // 2), (half + 1) * (DC // 2)),
                         ps_bufs=4)
                allgather(kin, kout)

                # V: rows x 512 features of this half, + interleaved ones.
                # Weight tile loaded once per kc; 4 row-chunk psums accumulate.
                psvs = [psum_pool.tile([128, 512], F32, tag="mm", bufs=4,
                                       name=f"psv_{name}_{rc}")
                        for rc in range(R // 128)]
                for kc in range(DC):
                    wt = pool.tile([128, 512], BF16, tag="wv",
                                   name=f"wv_{name}_{kc}")
                    nc.sync.dma_start(wt[:], wv_d.ap()[half, kc])
                    for rc in range(R // 128):
                        nc.tensor.matmul(
                            psvs[rc][:],
                            lhsT=src_sb[:, kc, rc * 128:(rc + 1) * 128],
                            rhs=wt[:], start=(kc == 0), stop=False)
                for rc in range(R // 128):
                    nc.tensor.matmul(psvs[rc][:], lhsT=ones_rbf[:],
                                     rhs=bv_row[:, half * 512:(half + 1) * 512],
                                     start=False, stop=True)
                    vl = pool.tile([128, 8, HD + 1], BF16, tag="vcopy",
                                   name=f"vl_{name}_{rc}")
                    nc.vector.tensor_copy(
                        vl[:, :, 0:HD],
                        psvs[rc][:].rearrange("p (h d) -> p h d", d=HD))
                    nc.vector.memset(vl[:, :, HD:HD + 1], 1.0)
                    nc.sync.dma_start(
                        vin[rc * 128:(rc + 1) * 128, :],
                        vl[:].rearrange("p h d -> p (h d)"))
                allgather(vin, vout)

            def attention(qt, kouts, vouts, dst_sb, masked, pool, psum_pool,
                          name):
                """dst_sb [128, DC, R] bf16 = attn(Q^T, gathered K/V)^T."""
                for hp in range(NPAIR):
                    half, hpl = hp // (NPAIR // 2), hp % (NPAIR // 2)
                    ko, vo = kouts[half], vouts[half]
                    # K^T tile: [feat 128, kc, key 128]
                    ktp = pool.tile([128, KC, 128], BF16, tag="ktp",
                                    name=f"kt_{name}_{hp}")
                    vt = pool.tile([128, KC, 2 * (HD + 1)], BF16, tag="vt",
                                   name=f"v_{name}_{hp}")
                    j0 = hpl * 2 * (HD + 1)
                    for s in range(4):
                        f0 = 512 * s + 128 * hpl
                        nc.sync.dma_start(
                            ktp[:, 4 * s:4 * s + 4, :],
                            ko[f0:f0 + 128, :]
                            .rearrange("p (c m) -> p c m", m=128))
                        # V tile: [key 128, kc, 2*(HD+1)] heads + ones cols
                        nc.sync.dma_start(
                            vt[:, 4 * s:4 * s + 4, :],
                            vo[512 * s:512 * s + 512, j0:j0 + 2 * (HD + 1)]
                            .rearrange("(c p) w -> p c w", p=128))
                    psA = psum_pool.tile([HD + 1, R], F32, tag="psO", bufs=2,
                                         name=f"oA_{name}_{hp}")
                    psB = psum_pool.tile([HD + 1, R], F32, tag="psO", bufs=2,
                                         name=f"oB_{name}_{hp}")
                    for kc in range(KC):
                        pss = psum_pool.tile([128, 2, R], F32, tag="psS", bufs=3,
                                             name=f"s_{name}_{hp}_{kc}")
                        nc.tensor.matmul(pss[:, 0, :],
                                         lhsT=ktp[0:64, kc, :],
                                         rhs=qt[0:64, hp, :],
                                         start=True, stop=True)
                        nc.tensor.matmul(pss[:, 1, :],
                                         lhsT=ktp[64:128, kc, :],
                                         rhs=qt[64:128, hp, :],
                                         start=True, stop=True)
                        es = pool.tile([128, 2, R], BF16, tag="expS", bufs=3,
                                       name=f"e_{name}_{hp}_{kc}")
                        nc.scalar.activation(
                            es[:].rearrange("p a f -> p (a f)"),
                            pss[:].rearrange("p a f -> p (a f)"),
                            AFT.Exp, scale=SCALE)
                        if masked:
                            nc.vector.tensor_tensor(
                                es[:], es[:],
                                mask_sb[:, kc, None, :].to_broadcast((128, 2, R)),
                                mybir.AluOpType.mult)
                        nc.tensor.matmul(psA[:], lhsT=vt[:, kc, 0:HD + 1],
                                         rhs=es[:, 0, :],
                                         start=(kc == 0), stop=(kc == KC - 1))
                        nc.tensor.matmul(psB[:], lhsT=vt[:, kc, HD + 1:],
                                         rhs=es[:, 1, :],
                                         start=(kc == 0), stop=(kc == KC - 1))
                    # normalize: copy raw denom, PE-broadcast to 64 lanes,
                    # reciprocal, multiply
                    rec = pool.tile([HD + 1, R], F32, tag="rec", bufs=4,
                                    name=f"r_{name}_{hp}")
                    nc.vector.tensor_copy(rec[HD:HD + 1, 0:R],
                                          psA[HD:HD + 1, :])
                    recB = pool.tile([HD + 1, R], F32, tag="recB", bufs=4,
                                     name=f"rB_{name}_{hp}")
                    nc.vector.tensor_copy(recB[HD:HD + 1, 0:R],
                                          psB[HD:HD + 1, :])
                    psn = psum_pool.tile([128, 2, R], F32, tag="psS", bufs=3,
                                         name=f"n_{name}_{hp}")
                    nc.tensor.matmul(psn[0:HD, 0, :],
                                     lhsT=ones_f32[HD:HD + 1, 0:HD],
                                     rhs=rec[HD:HD + 1, :],
                                     start=True, stop=True)
                    nc.tensor.matmul(psn[0:HD, 1, :],
                                     lhsT=ones_f32[HD:HD + 1, 0:HD],
                                     rhs=recB[HD:HD + 1, :],
                                     start=True, stop=True)
                    bcs = pool.tile([HD, 2, R], F32, tag="bcs",
                                    name=f"c_{name}_{hp}")
                    nc.vector.reciprocal_approx_fast(bcs[:, 0, :],
                                                     psn[0:HD, 0, :])
                    nc.vector.reciprocal_approx_fast(bcs[:, 1, :],
                                                     psn[0:HD, 1, :])
                    nc.vector.tensor_tensor(
                        dst_sb[0:HD, hp, :], psA[0:HD, :], bcs[:, 0, :],
                        mybir.AluOpType.mult)
                    tmb = pool.tile([HD, R], BF16, tag="tmb",
                                    name=f"t_{name}_{hp}")
                    nc.vector.tensor_tensor(tmb[:], psB[0:HD, :],
                                            bcs[:, 1, :],
                                            mybir.AluOpType.mult)
                    nc.sync.dma_start(dst_sb[HD:128, hp, :], tmb[:])

            # ============ phase 1: ln1, K/V(+gathers), Q, cross K/V ==========
            with (
                tc.tile_pool(name="p1", bufs=2) as pool,
                tc.tile_pool(name="p1ps", bufs=2, space="PSUM") as psum_pool,
            ):
                layernorm(x_sb, ln1_sb, pool, psum_pool, "ln1")
                kv_half(ln1_sb, wk, wv, B_K, bv_sb, kin_s[0], kout_s[0],
                        vin_s[0], vout_s[0], pool, psum_pool, 0, "sA")
                kv_half(ln1_sb, wk, wv, B_K, bv_sb, kin_s[1], kout_s[1],
                        vin_s[1], vout_s[1], pool, psum_pool, 1, "sB")

                def eat_q(oc, ps):
                    nc.scalar.activation(qt_sb[:, oc, :], ps[:], AFT.Identity,
                                         bias=bias_ap(B_Q, oc))

                matmul_t(ln1_sb, wq, DC, DC, pool, psum_pool, "q", eat_q,
                         ps_bufs=4)

                kv_half(enc_sb, wkvk, wkvv, B_KVK, bkvv_sb, kin_c[0],
                        kout_c[0], vin_c[0], vout_c[0], pool, psum_pool,
                        0, "cA")
                kv_half(enc_sb, wkvk, wkvv, B_KVK, bkvv_sb, kin_c[1],
                        kout_c[1], vin_c[1], vout_c[1], pool, psum_pool,
                        1, "cB")

            nc.sync.dma_start(mask_sb[:], mask_t.ap())
            if dbg:
                nc.sync.dma_start(dbg_t["ln1"].ap(), ln1_sb[:])
                nc.sync.dma_start(dbg_t["qt"].ap(), qt_sb[:])

            # ============ phase 2: self attention ============================
            with (
                tc.tile_pool(name="p2", bufs=2) as pool,
                tc.tile_pool(name="p2ps", bufs=2, space="PSUM") as psum_pool,
            ):
                attention(qt_sb, kout_s, vout_s, at_sb, True, pool, psum_pool,
                          "sa")
            if dbg:
                nc.sync.dma_start(dbg_t["at"].ap(), at_sb[:])

            # ============ phase 3: proj + residual, ln2, q2 ==================
            with (
                tc.tile_pool(name="p3", bufs=2) as pool,
                tc.tile_pool(name="p3ps", bufs=2, space="PSUM") as psum_pool,
            ):
                def eat_proj(oc, ps):
                    nc.vector.scalar_tensor_tensor(
                        x2_sb[:, oc, :], ps[:], bias_ap(B_PROJ, oc),
                        x_sb[:, oc, :],
                        mybir.AluOpType.add, mybir.AluOpType.add)

                matmul_t(at_sb, wproj, DC, DC, pool, psum_pool, "pr", eat_proj)
                if dbg:
                    nc.sync.dma_start(dbg_t["x2"].ap(), x2_sb[:])
                layernorm(x2_sb, ln1_sb, pool, psum_pool, "ln2")

                def eat_q2(oc, ps):
                    nc.scalar.activation(qt2_sb[:, oc, :], ps[:], AFT.Identity,
                                         bias=bias_ap(B_Q2, oc))

                matmul_t(ln1_sb, wq2, DC, DC, pool, psum_pool, "q2", eat_q2)

            # ============ phase 4: cross attention ===========================
            with (
                tc.tile_pool(name="p4", bufs=2) as pool,
                tc.tile_pool(name="p4ps", bufs=2, space="PSUM") as psum_pool,
            ):
                attention(qt2_sb, kout_c, vout_c, at_sb, False, pool,
                          psum_pool, "ca")

            # ============ phase 5: co + residual, ln3, MLP ===================
            with (
                tc.tile_pool(name="p5", bufs=2) as pool,
                tc.tile_pool(name="p5ps", bufs=2, space="PSUM") as psum_pool,
            ):
                def eat_co(oc, ps):
                    nc.vector.scalar_tensor_tensor(
                        x_sb[:, oc, :], ps[:], bias_ap(B_CO, oc),
                        x2_sb[:, oc, :],
                        mybir.AluOpType.add, mybir.AluOpType.add)

                matmul_t(at_sb, wco, DC, DC, pool, psum_pool, "co", eat_co)
                layernorm(x_sb, ln1_sb, pool, psum_pool, "ln3")

                h_sb = pool.tile([128, MC, R], BF16, tag="hsb", bufs=1)

                def eat_m1(oc, ps):
                    nc.scalar.activation(h_sb[:, oc, :], ps[:], AFT.Gelu,
                                         bias=bias_ap(B_M1, oc))

                matmul_t(ln1_sb, wm1, DC, MC, pool, psum_pool, "m1", eat_m1)

                def eat_m2(oc, ps):
                    nc.vector.scalar_tensor_tensor(
                        x2_sb[:, oc, :], ps[:], bias_ap(B_M2, oc),
                        x_sb[:, oc, :],
                        mybir.AluOpType.add, mybir.AluOpType.add)
                    nc.sync.dma_start(out_t.ap()[:, oc, :], x2_sb[:, oc, :])

                matmul_t(h_sb, wm2, MC, DC, pool, psum_pool, "m2", eat_m2,
                         w_tag="wtile2")

    nc.finalize()
    return nc


def zig_rows(g):
    """Zigzag row assignment: core g owns [256g,256g+256) u [1024+256g,+256)."""
    return np.concatenate([np.arange(256 * g, 256 * g + 256),
                           np.arange(1024 + 256 * g, 1024 + 256 * g + 256)])


def prep_inputs(inputs):
    """Host-side prep: fold LN affine into weights, cast/tile, shard rows."""
    f32 = np.float32
    bf16 = ml_dtypes.bfloat16

    def tile_w(w, nk, no):
        # [nk*128, no*128] -> [no, 128, nk, 128] (contiguous per-oc tiles)
        return np.ascontiguousarray(
            w.reshape(nk, 128, no, 128).transpose(2, 1, 0, 3)).astype(bf16)

    def tile_v(w):
        # [D, D] -> [2, DC, 128, 512] (contiguous [128, 512] tiles per half)
        return np.ascontiguousarray(
            w.reshape(DC, 128, 2, 512).transpose(2, 0, 1, 3)).astype(bf16)

    def chunk_b(b, n):
        return np.ascontiguousarray(b.reshape(n, 128)).astype(f32)

    def chunk_t(a):
        # [rows, D] -> [128, DC, rows] transposed chunked
        return np.ascontiguousarray(
            a.T.reshape(DC, 128, -1).transpose(1, 0, 2))

    x = np.asarray(inputs["x"], f32)
    enc = np.asarray(inputs["enc_out"], f32)
    cm = np.asarray(inputs["causal_mask"])

    ln1_g, ln1_b = np.asarray(inputs["ln1_g"], f32), np.asarray(inputs["ln1_b"], f32)
    ln2_g, ln2_b = np.asarray(inputs["ln2_g"], f32), np.asarray(inputs["ln2_b"], f32)
    ln3_g, ln3_b = np.asarray(inputs["ln3_g"], f32), np.asarray(inputs["ln3_b"], f32)
    qkv_w = np.asarray(inputs["qkv_w"], f32)
    qkv_b = np.asarray(inputs["qkv_b"], f32)
    q_w, q_b = np.asarray(inputs["q_w"], f32), np.asarray(inputs["q_b"], f32)
    kv_w, kv_b = np.asarray(inputs["kv_w"], f32), np.asarray(inputs["kv_b"], f32)
    mlp1_w, mlp1_b = np.asarray(inputs["mlp1_w"], f32), np.asarray(inputs["mlp1_b"], f32)

    qkv_w_eff = ln1_g[:, None] * qkv_w
    qkv_b_eff = qkv_b + ln1_b @ qkv_w
    q_w_eff = ln2_g[:, None] * q_w
    q_b_eff = q_b + ln2_b @ q_w
    m1_w_eff = ln3_g[:, None] * mlp1_w
    m1_b_eff = mlp1_b + ln3_b @ mlp1_w

    shared = {
        "wq": tile_w(qkv_w_eff[:, 0:D], DC, DC),
        "wk": tile_w(qkv_w_eff[:, D:2 * D], DC, DC),
        "wv": tile_v(qkv_w_eff[:, 2 * D:3 * D]),
        "wproj": tile_w(np.asarray(inputs["proj_w"], f32), DC, DC),
        "wq2": tile_w(q_w_eff, DC, DC),
        "wkvk": tile_w(kv_w[:, 0:D], DC, DC),
        "wkvv": tile_v(kv_w[:, D:2 * D]),
        "wco": tile_w(np.asarray(inputs["co_w"], f32), DC, DC),
        "wm1": tile_w(m1_w_eff, DC, MC),
        "wm2": tile_w(np.asarray(inputs["mlp2_w"], f32), MC, DC),
        "bq": chunk_b(qkv_b_eff[0:D], DC),
        "bk": chunk_b(qkv_b_eff[D:2 * D], DC),
        "bv": qkv_b_eff[2 * D:3 * D].reshape(1, D).astype(bf16),
        "bproj": chunk_b(np.asarray(inputs["proj_b"], f32), DC),
        "bq2": chunk_b(q_b_eff, DC),
        "bkvk": chunk_b(kv_b[0:D], DC),
        "bkvv": kv_b[D:2 * D].reshape(1, D).astype(bf16),
        "bco": chunk_b(np.asarray(inputs["co_b"], f32), DC),
        "bm1": chunk_b(m1_b_eff, MC),
        "bm2": chunk_b(np.asarray(inputs["mlp2_b"], f32), DC),
    }

    mask_f = (cm != 0).astype(f32)
    in_maps = []
    for c in range(N_CORES):
        b = c // GROUP
        r0 = (c % GROUP) * R
        m = dict(shared)
        m["x_t"] = chunk_t(x[b, r0:r0 + R]).astype(f32)
        m["enc_t"] = chunk_t(enc[b, r0:r0 + R]).astype(bf16)
        m["mask_t"] = np.ascontiguousarray(
            mask_f[r0:r0 + R].T.reshape(KC, 128, R).transpose(1, 0, 2)
        ).astype(bf16)
        in_maps.append(m)
    return in_maps


_prog_cache = {}


def kernel(**inputs):
    if "nc" not in _prog_cache:
        _prog_cache["nc"] = build_program()
    nc = _prog_cache["nc"]
    in_maps = prep_inputs(inputs)
    res = run_bass_kernel_spmd(nc, in_maps, core_ids=list(range(N_CORES)))
    out = np.empty((B, T, D), np.float32)
    for c in range(N_CORES):
        b = c // GROUP
        r0 = (c % GROUP) * R
        # out_t [128, DC, R] -> [R, D]
        ot = res.results[c]["out_t"]
        out[b, r0:r0 + R] = ot.transpose(2, 1, 0).reshape(R, D)
    _prog_cache["last_results"] = res
    return out


# revision 52
# speedup vs baseline: 1.2472x; 1.0888x over previous
"""Trainium2 Bass kernel for nn_DecoderBlock (B=2, T=2048, D=1024, H=16, MLP=4096).

Sharding: sequence/row parallel over 8 cores (4 cores per batch, 512 rows each).
K/V for both attentions are computed on local rows and AllGathered (2 groups of
4 cores).  K and V are packed into ONE gather buffer per head-half (4 gathers
total), launched as early as possible so they overlap Q / cross-KV compute.

On-device layout: activations transposed [feature, row]; LayerNorm affine
params folded into the following weight matrix on the host.  Weights are
pre-tiled on the host so every SBUF weight tile is one contiguous DRAM block
(fat DMA descriptors).  Softmax without max-subtraction; mask applied as 0/1
multiply on exp(scores); denominator comes from a ones column interleaved with
V in the gather buffer ([r, 8*65] layout).  LN rstd via exp(-0.5*ln(var+eps))
so the whole kernel uses one ACT table set until the final Gelu.
"""

import sys

if "/opt/trn_rl_repo" not in sys.path:
    sys.path.insert(0, "/opt/trn_rl_repo")

import numpy as np
import ml_dtypes

import concourse.bass as bass
import concourse.mybir as mybir
import concourse.tile as tile
from concourse import bacc
from concourse.bass_utils import run_bass_kernel_spmd

F32 = mybir.dt.float32
BF16 = mybir.dt.bfloat16

B, T, D, H, HD = 2, 2048, 1024, 16, 64
MLP = 4 * D
EPS = 1e-5
N_CORES = 8
GROUP = 4            # cores per batch element
R = T // GROUP       # rows per core = 512
DC = D // 128        # feature chunks = 8
MC = MLP // 128      # mlp chunks = 32
KC = T // 128        # key chunks = 16
NPAIR = H // 2       # head pairs = 8
SCALE = HD ** -0.5
VW = 8 * (HD + 1)    # V gather row width (ones interleaved) = 520
GW = 512 + VW        # gather buffer row width = 1032
NB = 256             # row block (2 zigzag blocks per core)


def build_program(dbg=False):
    nc = bacc.Bacc("TRN2", target_bir_lowering=False, debug=False,
                   num_devices=N_CORES)
    dbg_t = {}
    if dbg:
        dbg_t["ln1"] = nc.dram_tensor("d_ln1", [128, DC, R], BF16,
                                      kind="ExternalOutput")
        dbg_t["qt"] = nc.dram_tensor("d_qt", [128, DC, R], BF16,
                                     kind="ExternalOutput")
        dbg_t["at"] = nc.dram_tensor("d_at", [128, DC, R], BF16,
                                     kind="ExternalOutput")
        dbg_t["x2"] = nc.dram_tensor("d_x2", [128, DC, R], F32,
                                     kind="ExternalOutput")
        dbg_t["pss0"] = nc.dram_tensor("d_pss0", [128, 2, R], F32,
                                       kind="ExternalOutput")
        dbg_t["es0"] = nc.dram_tensor("d_es0", [128, 2, R], BF16,
                                      kind="ExternalOutput")
        dbg_t["psA"] = nc.dram_tensor("d_psA", [HD + 1, R], F32,
                                      kind="ExternalOutput")
        dbg_t["psB"] = nc.dram_tensor("d_psB", [HD + 1, R], F32,
                                      kind="ExternalOutput")
        dbg_t["bcs"] = nc.dram_tensor("d_bcs", [HD, 2, R], F32,
                                      kind="ExternalOutput")

    # ---- DRAM I/O ----
    x_t = nc.dram_tensor("x_t", [128, DC, R], F32, kind="ExternalInput")
    enc_t = nc.dram_tensor("enc_t", [128, DC, R], BF16, kind="ExternalInput")
    mask_t = nc.dram_tensor("mask_t", [128, KC, R], BF16, kind="ExternalInput")
    # dense weights pre-tiled [n_out, 128, n_k, 128]
    wq = nc.dram_tensor("wq", [DC, 128, DC, 128], BF16, kind="ExternalInput")
    wk = nc.dram_tensor("wk", [DC, 128, DC, 128], BF16, kind="ExternalInput")
    wv = nc.dram_tensor("wv", [2, DC, 128, 512], BF16, kind="ExternalInput")
    wproj = nc.dram_tensor("wproj", [DC, 128, DC, 128], BF16, kind="ExternalInput")
    wq2 = nc.dram_tensor("wq2", [DC, 128, DC, 128], BF16, kind="ExternalInput")
    wkvk = nc.dram_tensor("wkvk", [DC, 128, DC, 128], BF16, kind="ExternalInput")
    wkvv = nc.dram_tensor("wkvv", [2, DC, 128, 512], BF16, kind="ExternalInput")
    wco = nc.dram_tensor("wco", [DC, 128, DC, 128], BF16, kind="ExternalInput")
    wm1 = nc.dram_tensor("wm1", [MC, 128, DC, 128], BF16, kind="ExternalInput")
    wm2 = nc.dram_tensor("wm2", [DC, 128, MC, 128], BF16, kind="ExternalInput")
    bq = nc.dram_tensor("bq", [DC, 128], F32, kind="ExternalInput")
    bk = nc.dram_tensor("bk", [DC, 128], F32, kind="ExternalInput")
    bv = nc.dram_tensor("bv", [1, D], BF16, kind="ExternalInput")
    bproj = nc.dram_tensor("bproj", [DC, 128], F32, kind="ExternalInput")
    bq2 = nc.dram_tensor("bq2", [DC, 128], F32, kind="ExternalInput")
    bkvk = nc.dram_tensor("bkvk", [DC, 128], F32, kind="ExternalInput")
    bkvv = nc.dram_tensor("bkvv", [1, D], BF16, kind="ExternalInput")
    bco = nc.dram_tensor("bco", [DC, 128], F32, kind="ExternalInput")
    bm1 = nc.dram_tensor("bm1", [MC, 128], F32, kind="ExternalInput")
    bm2 = nc.dram_tensor("bm2", [DC, 128], F32, kind="ExternalInput")
    out_t = nc.dram_tensor("out_t", [128, DC, R], F32, kind="ExternalOutput")

    rg = [[0, 1, 2, 3], [4, 5, 6, 7]]
    AFT = mybir.ActivationFunctionType

    with tile.TileContext(nc) as tc:
        with (
            tc.tile_pool(name="persist", bufs=1) as pp,
            tc.tile_pool(name="dram", bufs=1, space="DRAM") as dram,
        ):
            # gather buffers (<=520KB so collectives take the mesh path):
            # K^T half [512 feat, 512 tok]; V half [512 tok, 8*65] ones-interleaved
            kin_s = [dram.tile([512, 512], BF16, name=f"kinS{i}") for i in range(2)]
            kout_s = [dram.tile([2048, 512], BF16, name=f"koutS{i}") for i in range(2)]
            vin_s = [dram.tile([512, VW], BF16, name=f"vinS{i}") for i in range(2)]
            vout_s = [dram.tile([2048, VW], BF16, name=f"voutS{i}") for i in range(2)]
            kin_c = [dram.tile([512, 512], BF16, name=f"kinC{i}") for i in range(2)]
            kout_c = [dram.tile([2048, 512], BF16, name=f"koutC{i}") for i in range(2)]
            vin_c = [dram.tile([512, VW], BF16, name=f"vinC{i}") for i in range(2)]
            vout_c = [dram.tile([2048, VW], BF16, name=f"voutC{i}") for i in range(2)]

            def allgather(src_d, dst_d):
                nc.gpsimd.collective_compute(
                    "AllGather", mybir.AluOpType.bypass,
                    ins=[src_d.opt()], outs=[dst_d.opt()], replica_groups=rg)

            # persistent SBUF
            x_sb = pp.tile([128, DC, R], F32)        # residual stream x^T
            x2_sb = pp.tile([128, DC, R], F32)
            enc_sb = pp.tile([128, DC, R], BF16)
            mask_sb = pp.tile([128, KC, R], BF16)
            ln1_sb = pp.tile([128, DC, R], BF16)
            qt_sb = pp.tile([128, DC, R], BF16)      # Q^T (self)
            qt2_sb = pp.tile([128, DC, R], BF16)     # Q^T (cross)
            at_sb = pp.tile([128, DC, R], BF16)      # attn out^T (reused)
            ones_rbf = pp.tile([1, 128], BF16)       # K=1 lhsT for bf16 bcasts
            ones_f32 = pp.tile([128, 128], F32)      # K=1 lhsT rows for f32 bcasts
            ones_bf_col = pp.tile([128, 1], BF16)    # lhsT for partition sums
            bias_sb = pp.tile([128, 8 * DC + MC], F32)
            bv_sb = pp.tile([1, D], BF16)
            bkvv_sb = pp.tile([1, D], BF16)
            eps_sb = pp.tile([1, 1], F32)

            nc.vector.memset(ones_rbf[:], 1.0)
            nc.vector.memset(ones_f32[:], 1.0)
            nc.vector.memset(ones_bf_col[:], 1.0)
            nc.vector.memset(eps_sb[:], EPS)

            for c in range(DC):
                nc.sync.dma_start(x_sb[:, c, :], x_t.ap()[:, c, :])
                nc.sync.dma_start(enc_sb[:, c, :], enc_t.ap()[:, c, :])
            nc.sync.dma_start(bv_sb[:], bv.ap())
            nc.sync.dma_start(bkvv_sb[:], bkvv.ap())
            bias_list = [bq, bk, bproj, bq2, bkvk, bco, bm2]
            for i, b in enumerate(bias_list):
                nc.sync.dma_start(
                    bias_sb[:, i * DC:(i + 1) * DC],
                    b.ap().rearrange("c p -> p c"),
                )
            nc.sync.dma_start(
                bias_sb[:, 7 * DC:7 * DC + MC], bm1.ap().rearrange("c p -> p c")
            )
            B_Q, B_K, B_PROJ, B_Q2, B_KVK, B_CO, B_M2 = (
                0, DC, 2 * DC, 3 * DC, 4 * DC, 5 * DC, 6 * DC)
            B_M1 = 7 * DC

            def bias_ap(base, oc):
                return bias_sb[:, base + oc:base + oc + 1]

            # ---------- helpers ----------
            def layernorm(src_sb, dst_sb, pool, psum_pool, name,
                          cs=slice(0, R), stag="stats", btag="lnb", sbufs=2,
                          bbufs=1):
                """src_sb [128, DC, R] f32 cols cs -> ln^T bf16 into dst_sb."""
                n = cs.stop - cs.start
                ps1 = psum_pool.tile([1, n], F32, tag=stag, bufs=sbufs,
                                     name=f"p1_{name}")
                ps2 = psum_pool.tile([1, n], F32, tag=stag, bufs=sbufs,
                                     name=f"p2_{name}")
                for c in range(DC):
                    xb = pool.tile([128, R], BF16, tag="lnxb", name=f"xb_{name}{c}")
                    sq = pool.tile([128, R], BF16, tag="lnsq", name=f"sq_{name}{c}")
                    nc.vector.tensor_copy(xb[:, 0:n], src_sb[:, c, cs])
                    nc.vector.tensor_tensor(sq[:, 0:n], xb[:, 0:n], xb[:, 0:n],
                                            mybir.AluOpType.mult)
                    nc.tensor.matmul(ps1[0:1, 0:n], lhsT=ones_bf_col[:],
                                     rhs=xb[:, 0:n],
                                     start=(c == 0), stop=(c == DC - 1))
                    nc.tensor.matmul(ps2[0:1, 0:n], lhsT=ones_bf_col[:],
                                     rhs=sq[:, 0:n],
                                     start=(c == 0), stop=(c == DC - 1))
                nmean = pool.tile([1, R], F32, tag="lnrow", bufs=8, name=f"nm_{name}")
                ex2 = pool.tile([1, R], F32, tag="lnrow", bufs=8, name=f"e2_{name}")
                m2 = pool.tile([1, R], F32, tag="lnrow", bufs=8, name=f"m2_{name}")
                var = pool.tile([1, R], F32, tag="lnrow", bufs=8, name=f"va_{name}")
                lnv = pool.tile([1, R], F32, tag="lnrow", bufs=8, name=f"lv_{name}")
                rstd = pool.tile([1, R], F32, tag="lnrow", bufs=8, name=f"rs_{name}")
                nmrs = pool.tile([1, R], F32, tag="lnrow", bufs=8, name=f"nr_{name}")
                nc.scalar.activation(nmean[0:1, 0:n], ps1[0:1, 0:n], AFT.Identity,
                                     scale=-1.0 / D)
                nc.scalar.activation(ex2[0:1, 0:n], ps2[0:1, 0:n], AFT.Identity,
                                     scale=1.0 / D)
                nc.vector.tensor_tensor(m2[0:1, 0:n], nmean[0:1, 0:n],
                                        nmean[0:1, 0:n], mybir.AluOpType.mult)
                nc.vector.tensor_tensor(var[0:1, 0:n], ex2[0:1, 0:n],
                                        m2[0:1, 0:n], mybir.AluOpType.subtract)
                # rstd = exp(-0.5 * ln(var + eps)) — stays in the exp/ln set
                nc.scalar.activation(lnv[0:1, 0:n], var[0:1, 0:n], AFT.Ln,
                                     bias=eps_sb[:])
                nc.scalar.activation(rstd[0:1, 0:n], lnv[0:1, 0:n], AFT.Exp,
                                     scale=-0.5)
                nc.vector.tensor_tensor(nmrs[0:1, 0:n], nmean[0:1, 0:n],
                                        rstd[0:1, 0:n], mybir.AluOpType.mult)
                psb = psum_pool.tile([128, 2, n], F32, tag=btag, bufs=bbufs,
                                     name=f"pb_{name}")
                nc.tensor.matmul(psb[:, 0, 0:n], lhsT=ones_f32[0:1, :],
                                 rhs=rstd[0:1, 0:n], start=True, stop=True)
                nc.tensor.matmul(psb[:, 1, 0:n], lhsT=ones_f32[0:1, :],
                                 rhs=nmrs[0:1, 0:n], start=True, stop=True)
                tmp = pool.tile([128, R], F32, tag="lntmp", name=f"tp_{name}")
                for c in range(DC):
                    nc.vector.tensor_tensor(tmp[:, 0:n], src_sb[:, c, cs],
                                            psb[:, 0, 0:n],
                                            mybir.AluOpType.mult)
                    nc.vector.tensor_tensor(dst_sb[:, c, cs], tmp[:, 0:n],
                                            psb[:, 1, 0:n],
                                            mybir.AluOpType.add)

            def matmul_t(rhs_sb, w_dram, n_k, n_o, pool, psum_pool, name,
                         consume, w_tag="wtile", ocs=None, ps_bufs=3):
                """out^T[oc] = sum_kc W[oc][kc].T @ rhs[kc]; consume(oc, psum)."""
                for oc in (range(n_o) if ocs is None else ocs):
                    wt = pool.tile([128, n_k, 128], BF16, tag=w_tag, bufs=4,
                                   name=f"w_{name}_{oc}")
                    if n_k > 8:
                        h = n_k // 2
                        nc.sync.dma_start(wt[:, 0:h, :],
                                          w_dram.ap()[oc, :, 0:h, :])
                        nc.gpsimd.dma_start(wt[:, h:, :],
                                            w_dram.ap()[oc, :, h:, :])
                    else:
                        nc.sync.dma_start(wt[:], w_dram.ap()[oc])
                    ps = psum_pool.tile([128, R], F32, tag="mm", bufs=ps_bufs,
                                        name=f"ps_{name}_{oc}")
                    for kc in range(n_k):
                        nc.tensor.matmul(ps[:], lhsT=wt[:, kc, :],
                                         rhs=rhs_sb[:, kc, :],
                                         start=(kc == 0), stop=(kc == n_k - 1))
                    consume(oc, ps)

            def kv_half(src_sb, wk_d, wv_d, bk_base, bv_row, kin, kout, vin,
                        vout, pool, psum_pool, half, name):
                """K^T half + V half (with ones cols) -> gather buffers + AGs."""
                # K^T: 4 output chunks = features [512*half, 512*half+512)
                def eat_k(oc, ps):
                    ocl = oc - half * (DC // 2)
                    kl = pool.tile([128, R], BF16, tag="kvcopy", name=f"kl_{name}{oc}")
                    nc.vector.tensor_scalar_add(kl[:], ps[:], bias_ap(bk_base, oc))
                    nc.sync.dma_start(
                        kin[ocl * 128:(ocl + 1) * 128, :], kl[:])

                matmul_t(src_sb, wk_d, DC, DC, pool, psum_pool, f"k{name}",
                         eat_k, ocs=range(half * (DC // 2), (half + 1) * (DC // 2)),
                         ps_bufs=4)
                allgather(kin, kout)

                # V: rows x 512 features of this half, + interleaved ones.
                # Weight tile loaded once per kc; 4 row-chunk psums accumulate.
                psvs = [psum_pool.tile([128, 512], F32, tag="mm", bufs=4,
                                       name=f"psv_{name}_{rc}")
                        for rc in range(R // 128)]
                for kc in range(DC):
                    wt = pool.tile([128, 512], BF16, tag="wv", bufs=4,
                                   name=f"wv_{name}_{kc}")
                    nc.sync.dma_start(wt[:], wv_d.ap()[half, kc])
                    for rc in range(R // 128):
                        nc.tensor.matmul(
                            psvs[rc][:],
                            lhsT=src_sb[:, kc, rc * 128:(rc + 1) * 128],
                            rhs=wt[:], start=(kc == 0), stop=False)
                for rc in range(R // 128):
                    nc.tensor.matmul(psvs[rc][:], lhsT=ones_rbf[:],
                                     rhs=bv_row[:, half * 512:(half + 1) * 512],
                                     start=False, stop=True)
                    vl = pool.tile([128, 8, HD + 1], BF16, tag="vcopy",
                                   name=f"vl_{name}_{rc}")
                    nc.vector.tensor_copy(
                        vl[:, :, 0:HD],
                        psvs[rc][:].rearrange("p (h d) -> p h d", d=HD))
                    nc.vector.memset(vl[:, :, HD:HD + 1], 1.0)
                    nc.sync.dma_start(
                        vin[rc * 128:(rc + 1) * 128, :],
                        vl[:].rearrange("p h d -> p (h d)"))
                allgather(vin, vout)

            def attention(qt, kouts, vouts, dst_sb, masked, pool, psum_pool,
                          name):
                """dst_sb [128, DC, R] bf16 = attn(Q^T, gathered K/V)^T."""
                for hp in range(NPAIR):
                    half, hpl = hp // (NPAIR // 2), hp % (NPAIR // 2)
                    ko, vo = kouts[half], vouts[half]
                    # K^T tile: [feat 128, kc, key 128]
                    ktp = pool.tile([128, KC, 128], BF16, tag="ktp",
                                    name=f"kt_{name}_{hp}")
                    vt = pool.tile([128, KC, 2 * (HD + 1)], BF16, tag="vt",
                                   name=f"v_{name}_{hp}")
                    j0 = hpl * 2 * (HD + 1)
                    for s in range(4):
                        f0 = 512 * s + 128 * hpl
                        nc.gpsimd.dma_start(
                            ktp[:, 4 * s:4 * s + 4, :],
                            ko[f0:f0 + 128, :]
                            .rearrange("p (c m) -> p c m", m=128))
                        # V tile: [key 128, kc, 2*(HD+1)] heads + ones cols
                        nc.gpsimd.dma_start(
                            vt[:, 4 * s:4 * s + 4, :],
                            vo[512 * s:512 * s + 512, j0:j0 + 2 * (HD + 1)]
                            .rearrange("(c p) w -> p c w", p=128))
                    psA = psum_pool.tile([HD + 1, R], F32, tag="psO", bufs=2,
                                         name=f"oA_{name}_{hp}")
                    psB = psum_pool.tile([HD + 1, R], F32, tag="psO", bufs=2,
                                         name=f"oB_{name}_{hp}")
                    for kc in range(KC):
                        pss = psum_pool.tile([128, 2, R], F32, tag="psS", bufs=3,
                                             name=f"s_{name}_{hp}_{kc}")
                        nc.tensor.matmul(pss[:, 0, :],
                                         lhsT=ktp[0:64, kc, :],
                                         rhs=qt[0:64, hp, :],
                                         start=True, stop=True)
                        nc.tensor.matmul(pss[:, 1, :],
                                         lhsT=ktp[64:128, kc, :],
                                         rhs=qt[64:128, hp, :],
                                         start=True, stop=True)
                        es = pool.tile([128, 2, R], BF16, tag="expS", bufs=3,
                                       name=f"e_{name}_{hp}_{kc}")
                        nc.scalar.activation(
                            es[:].rearrange("p a f -> p (a f)"),
                            pss[:].rearrange("p a f -> p (a f)"),
                            AFT.Exp, scale=SCALE)
                        if masked:
                            nc.vector.tensor_tensor(
                                es[:], es[:],
                                mask_sb[:, kc, None, :].to_broadcast((128, 2, R)),
                                mybir.AluOpType.mult)
                        nc.tensor.matmul(psA[:], lhsT=vt[:, kc, 0:HD + 1],
                                         rhs=es[:, 0, :],
                                         start=(kc == 0), stop=(kc == KC - 1))
                        nc.tensor.matmul(psB[:], lhsT=vt[:, kc, HD + 1:],
                                         rhs=es[:, 1, :],
                                         start=(kc == 0), stop=(kc == KC - 1))
                    # normalize: copy raw denom, PE-broadcast to 64 lanes,
                    # reciprocal, multiply
                    rec = pool.tile([HD + 1, R], F32, tag="rec", bufs=4,
                                    name=f"r_{name}_{hp}")
                    nc.vector.tensor_copy(rec[HD:HD + 1, 0:R],
                                          psA[HD:HD + 1, :])
                    recB = pool.tile([HD + 1, R], F32, tag="recB", bufs=4,
                                     name=f"rB_{name}_{hp}")
                    nc.vector.tensor_copy(recB[HD:HD + 1, 0:R],
                                          psB[HD:HD + 1, :])
                    psn = psum_pool.tile([128, 2, R], F32, tag="psS", bufs=3,
                                         name=f"n_{name}_{hp}")
                    nc.tensor.matmul(psn[0:HD, 0, :],
                                     lhsT=ones_f32[HD:HD + 1, 0:HD],
                                     rhs=rec[HD:HD + 1, :],
                                     start=True, stop=True)
                    nc.tensor.matmul(psn[0:HD, 1, :],
                                     lhsT=ones_f32[HD:HD + 1, 0:HD],
                                     rhs=recB[HD:HD + 1, :],
                                     start=True, stop=True)
                    bcs = pool.tile([HD, 2, R], F32, tag="bcs",
                                    name=f"c_{name}_{hp}")
                    nc.vector.reciprocal_approx_fast(bcs[:, 0, :],
                                                     psn[0:HD, 0, :])
                    nc.vector.reciprocal_approx_fast(bcs[:, 1, :],
                                                     psn[0:HD, 1, :])
                    nc.vector.tensor_tensor(
                        dst_sb[0:HD, hp, :], psA[0:HD, :], bcs[:, 0, :],
                        mybir.AluOpType.mult)
                    tmb = pool.tile([HD, R], BF16, tag="tmb",
                                    name=f"t_{name}_{hp}")
                    nc.vector.tensor_tensor(tmb[:], psB[0:HD, :],
                                            bcs[:, 1, :],
                                            mybir.AluOpType.mult)
                    nc.sync.dma_start(dst_sb[HD:128, hp, :], tmb[:])

            # ============ phase 1: ln1, K/V(+gathers), Q, cross K/V ==========
            with (
                tc.tile_pool(name="p1", bufs=2) as pool,
                tc.tile_pool(name="p1ps", bufs=2, space="PSUM") as psum_pool,
            ):
                layernorm(x_sb, ln1_sb, pool, psum_pool, "ln1")
                kv_half(ln1_sb, wk, wv, B_K, bv_sb, kin_s[0], kout_s[0],
                        vin_s[0], vout_s[0], pool, psum_pool, 0, "sA")
                kv_half(ln1_sb, wk, wv, B_K, bv_sb, kin_s[1], kout_s[1],
                        vin_s[1], vout_s[1], pool, psum_pool, 1, "sB")

                def eat_q(oc, ps):
                    nc.scalar.activation(qt_sb[:, oc, :], ps[:], AFT.Identity,
                                         bias=bias_ap(B_Q, oc))

                matmul_t(ln1_sb, wq, DC, DC, pool, psum_pool, "q", eat_q,
                         ps_bufs=4)

                kv_half(enc_sb, wkvk, wkvv, B_KVK, bkvv_sb, kin_c[0],
                        kout_c[0], vin_c[0], vout_c[0], pool, psum_pool,
                        0, "cA")
                kv_half(enc_sb, wkvk, wkvv, B_KVK, bkvv_sb, kin_c[1],
                        kout_c[1], vin_c[1], vout_c[1], pool, psum_pool,
                        1, "cB")

            nc.sync.dma_start(mask_sb[:], mask_t.ap())
            if dbg:
                nc.sync.dma_start(dbg_t["ln1"].ap(), ln1_sb[:])
                nc.sync.dma_start(dbg_t["qt"].ap(), qt_sb[:])

            # ============ phase 2: self attention ============================
            with (
                tc.tile_pool(name="p2", bufs=2) as pool,
                tc.tile_pool(name="p2ps", bufs=2, space="PSUM") as psum_pool,
            ):
                attention(qt_sb, kout_s, vout_s, at_sb, True, pool, psum_pool,
                          "sa")
            if dbg:
                nc.sync.dma_start(dbg_t["at"].ap(), at_sb[:])

            # ============ phase 3: proj + residual, ln2, q2 ==================
            with (
                tc.tile_pool(name="p3", bufs=2) as pool,
                tc.tile_pool(name="p3ps", bufs=2, space="PSUM") as psum_pool,
            ):
                def eat_proj(oc, ps):
                    nc.vector.scalar_tensor_tensor(
                        x2_sb[:, oc, :], ps[:], bias_ap(B_PROJ, oc),
                        x_sb[:, oc, :],
                        mybir.AluOpType.add, mybir.AluOpType.add)

                matmul_t(at_sb, wproj, DC, DC, pool, psum_pool, "pr", eat_proj)
                if dbg:
                    nc.sync.dma_start(dbg_t["x2"].ap(), x2_sb[:])
                layernorm(x2_sb, ln1_sb, pool, psum_pool, "ln2")

                def eat_q2(oc, ps):
                    nc.scalar.activation(qt2_sb[:, oc, :], ps[:], AFT.Identity,
                                         bias=bias_ap(B_Q2, oc))

                matmul_t(ln1_sb, wq2, DC, DC, pool, psum_pool, "q2", eat_q2)

            # ============ phase 4: cross attention ===========================
            with (
                tc.tile_pool(name="p4", bufs=2) as pool,
                tc.tile_pool(name="p4ps", bufs=2, space="PSUM") as psum_pool,
            ):
                attention(qt2_sb, kout_c, vout_c, at_sb, False, pool,
                          psum_pool, "ca")

            # ============ phase 5: co + residual, ln3, MLP ===================
            with (
                tc.tile_pool(name="p5", bufs=2) as pool,
                tc.tile_pool(name="p5ps", bufs=2, space="PSUM") as psum_pool,
            ):
                def eat_co(oc, ps):
                    nc.vector.scalar_tensor_tensor(
                        x_sb[:, oc, :], ps[:], bias_ap(B_CO, oc),
                        x2_sb[:, oc, :],
                        mybir.AluOpType.add, mybir.AluOpType.add)

                matmul_t(at_sb, wco, DC, DC, pool, psum_pool, "co", eat_co)
                layernorm(x_sb, ln1_sb, pool, psum_pool, "ln3")

                h_sb = pool.tile([128, MC, R], BF16, tag="hsb", bufs=1)

                def eat_m1(oc, ps):
                    nc.scalar.activation(h_sb[:, oc, :], ps[:], AFT.Gelu,
                                         bias=bias_ap(B_M1, oc))

                matmul_t(ln1_sb, wm1, DC, MC, pool, psum_pool, "m1", eat_m1)

                def eat_m2(oc, ps):
                    nc.vector.scalar_tensor_tensor(
                        x2_sb[:, oc, :], ps[:], bias_ap(B_M2, oc),
                        x_sb[:, oc, :],
                        mybir.AluOpType.add, mybir.AluOpType.add)
                    nc.sync.dma_start(out_t.ap()[:, oc, :], x2_sb[:, oc, :])

                matmul_t(h_sb, wm2, MC, DC, pool, psum_pool, "m2", eat_m2,
                         w_tag="wtile2")

    nc.finalize()
    return nc


def zig_rows(g):
    """Zigzag row assignment: core g owns [256g,256g+256) u [1024+256g,+256)."""
    return np.concatenate([np.arange(256 * g, 256 * g + 256),
                           np.arange(1024 + 256 * g, 1024 + 256 * g + 256)])


def prep_inputs(inputs):
    """Host-side prep: fold LN affine into weights, cast/tile, shard rows."""
    f32 = np.float32
    bf16 = ml_dtypes.bfloat16

    def tile_w(w, nk, no):
        # [nk*128, no*128] -> [no, 128, nk, 128] (contiguous per-oc tiles)
        return np.ascontiguousarray(
            w.reshape(nk, 128, no, 128).transpose(2, 1, 0, 3)).astype(bf16)

    def tile_v(w):
        # [D, D] -> [2, DC, 128, 512] (contiguous [128, 512] tiles per half)
        return np.ascontiguousarray(
            w.reshape(DC, 128, 2, 512).transpose(2, 0, 1, 3)).astype(bf16)

    def chunk_b(b, n):
        return np.ascontiguousarray(b.reshape(n, 128)).astype(f32)

    def chunk_t(a):
        # [rows, D] -> [128, DC, rows] transposed chunked
        return np.ascontiguousarray(
            a.T.reshape(DC, 128, -1).transpose(1, 0, 2))

    x = np.asarray(inputs["x"], f32)
    enc = np.asarray(inputs["enc_out"], f32)
    cm = np.asarray(inputs["causal_mask"])

    ln1_g, ln1_b = np.asarray(inputs["ln1_g"], f32), np.asarray(inputs["ln1_b"], f32)
    ln2_g, ln2_b = np.asarray(inputs["ln2_g"], f32), np.asarray(inputs["ln2_b"], f32)
    ln3_g, ln3_b = np.asarray(inputs["ln3_g"], f32), np.asarray(inputs["ln3_b"], f32)
    qkv_w = np.asarray(inputs["qkv_w"], f32)
    qkv_b = np.asarray(inputs["qkv_b"], f32)
    q_w, q_b = np.asarray(inputs["q_w"], f32), np.asarray(inputs["q_b"], f32)
    kv_w, kv_b = np.asarray(inputs["kv_w"], f32), np.asarray(inputs["kv_b"], f32)
    mlp1_w, mlp1_b = np.asarray(inputs["mlp1_w"], f32), np.asarray(inputs["mlp1_b"], f32)

    qkv_w_eff = ln1_g[:, None] * qkv_w
    qkv_b_eff = qkv_b + ln1_b @ qkv_w
    q_w_eff = ln2_g[:, None] * q_w
    q_b_eff = q_b + ln2_b @ q_w
    m1_w_eff = ln3_g[:, None] * mlp1_w
    m1_b_eff = mlp1_b + ln3_b @ mlp1_w

    shared = {
        "wq": tile_w(qkv_w_eff[:, 0:D], DC, DC),
        "wk": tile_w(qkv_w_eff[:, D:2 * D], DC, DC),
        "wv": tile_v(qkv_w_eff[:, 2 * D:3 * D]),
        "wproj": tile_w(np.asarray(inputs["proj_w"], f32), DC, DC),
        "wq2": tile_w(q_w_eff, DC, DC),
        "wkvk": tile_w(kv_w[:, 0:D], DC, DC),
        "wkvv": tile_v(kv_w[:, D:2 * D]),
        "wco": tile_w(np.asarray(inputs["co_w"], f32), DC, DC),
        "wm1": tile_w(m1_w_eff, DC, MC),
        "wm2": tile_w(np.asarray(inputs["mlp2_w"], f32), MC, DC),
        "bq": chunk_b(qkv_b_eff[0:D], DC),
        "bk": chunk_b(qkv_b_eff[D:2 * D], DC),
        "bv": qkv_b_eff[2 * D:3 * D].reshape(1, D).astype(bf16),
        "bproj": chunk_b(np.asarray(inputs["proj_b"], f32), DC),
        "bq2": chunk_b(q_b_eff, DC),
        "bkvk": chunk_b(kv_b[0:D], DC),
        "bkvv": kv_b[D:2 * D].reshape(1, D).astype(bf16),
        "bco": chunk_b(np.asarray(inputs["co_b"], f32), DC),
        "bm1": chunk_b(m1_b_eff, MC),
        "bm2": chunk_b(np.asarray(inputs["mlp2_b"], f32), DC),
    }

    mask_f = (cm != 0).astype(f32)
    in_maps = []
    for c in range(N_CORES):
        b = c // GROUP
        r0 = (c % GROUP) * R
        m = dict(shared)
        m["x_t"] = chunk_t(x[b, r0:r0 + R]).astype(f32)
        m["enc_t"] = chunk_t(enc[b, r0:r0 + R]).astype(bf16)
        m["mask_t"] = np.ascontiguousarray(
            mask_f[r0:r0 + R].T.reshape(KC, 128, R).transpose(1, 0, 2)
        ).astype(bf16)
        in_maps.append(m)
    return in_maps


_prog_cache = {}


def kernel(**inputs):
    if "nc" not in _prog_cache:
        _prog_cache["nc"] = build_program()
    nc = _prog_cache["nc"]
    in_maps = prep_inputs(inputs)
    res = run_bass_kernel_spmd(nc, in_maps, core_ids=list(range(N_CORES)))
    out = np.empty((B, T, D), np.float32)
    for c in range(N_CORES):
        b = c // GROUP
        r0 = (c % GROUP) * R
        # out_t [128, DC, R] -> [R, D]
        ot = res.results[c]["out_t"]
        out[b, r0:r0 + R] = ot.transpose(2, 1, 0).reshape(R, D)
    _prog_cache["last_results"] = res
    return out
